# revision 35
# baseline (speedup 1.0000x reference)
"""Trainium2 Bass kernel for nn_CLIP_GCN_Model (2-layer GCN + MLP + contrastive loss).

Reformulation (validated numerically):
  out = mean_i(label_i * (lse_i - logits_ii)) + 1.0
(the triplet term of the reference is identically 1.0).

GCN layer: out = S @ (x @ W) + b where S = D^-1/2 (A+I) D^-1/2.

Structure (single collective, eager gather streams):
  1. L1 runs over all 10240 (padded) nodes: 80 dst-chunks of 128 nodes,
     balanced to (core, slot); per chunk the distinct source x rows (512B fp8)
     are dma_gathered and aggregated with a fp8 coefficient matrix C
     (DoubleRow matmuls in PSUM), then transposed, x W_g1 + bias + relu -> h.
     All 20 gather pieces (2 per slot, <=9 tiles each to fit the SWDGE ring)
     are dispatched up front, round-robined over the 4 queues so all four
     descriptor generators run concurrently.
  2. h slots are written p-major ([128, 10, 256] per rank; node row =
     rank*1280 + p*10 + slot) and shared with ONE AllGather -> h_t.
  3. The image MLP is fully replicated: every core encodes ALL 4096 images
     (fp8, column-rolled so its own 512 images sit in block 0 -- the row-wise
     LSE is invariant to logits column order). It is emitted AFTER the h
     AllGather trigger so TensorE fills the otherwise-idle mesh-wait window,
     and it removes the image AllGather entirely. A small warm-up block ramps
     the PE p-state / scalar activation tables before the first L1 slot.
  4. W_g2 is fused into the image side: M = W_g2-contracted image encodings
     [256, 4096] is computed once per core in the AllGather window, so each
     128-label row tile only needs its h aggregation [128, 256], a transpose,
     and ONE fp8 DoubleRow matmul per 512-column tile (K=256 instead of 512).
     The diagonal is extracted as diag(a2t^T @ M_own) via identity mask +
     row reduce. LSE uses fused exp+accumulate (fast path: a host-side bound
     check shows exp cannot overflow in this data regime).
"""

import os
import numpy as np
import ml_dtypes

BF16 = ml_dtypes.bfloat16
F8 = ml_dtypes.float8_e4m3   # TRN fp8e4 (max 240)

N_NODES = 10000
NPAD = 10240
D = 512
Hdim = 256
BATCH = 4096
NCORES = 8
P = 128
NCHUNK = NPAD // P          # 80
CPC = NCHUNK // NCORES      # 10 slots per core
NPC = NPAD // NCORES        # 1280 nodes per core
MPC = NPC // P              # 10 m-tiles per core
RT = 4                      # row tiles per core (512 rows each core)
NT = BATCH // 512           # 8 column tiles of 512
H5 = CPC // 2               # 5 slots per h-half
HALF_N = NPAD // 2          # 5120


def _wrap16(idx, n):
    """Layout indices for dma_gather: element i -> [i%16, i//16], replicated to 128 partitions."""
    assert len(idx) == n and n % 16 == 0
    base = idx.astype(np.int16).reshape(n // 16, 16).T  # [16, n/16]
    return np.ascontiguousarray(np.tile(base, (8, 1)))  # [128, n/16]


def _prep(inputs):
    """Host-side layout/sharding prep."""
    x = np.asarray(inputs["x_nodes"], dtype=np.float32)
    image = np.asarray(inputs["image"], dtype=np.float32)
    ei = np.asarray(inputs["edge_index"]).astype(np.int64)
    label = np.asarray(inputs["label"]).astype(np.int64)
    src, dst = ei[0], ei[1]

    deg = np.ones(N_NODES, np.float32)
    np.add.at(deg, dst, 1.0)
    dinv = (1.0 / np.sqrt(deg)).astype(np.float32)

    # in-edges grouped by dst (sorted once)
    order = np.argsort(dst, kind="stable")
    src_s, dst_s = src[order], dst[order]
    bound = np.searchsorted(dst_s, np.arange(N_NODES + 1))

    nn = np.arange(NPAD)

    # ---------------- L1: per-chunk dedup + balanced (core,slot) assignment --
    chunk_src = []      # distinct sources per chunk
    chunk_C = []        # [n_distinct, 128] fp32 coef
    for c in range(NCHUNK):
        n0, n1 = c * P, min((c + 1) * P, N_NODES)
        if n0 >= N_NODES:
            chunk_src.append(np.zeros(1, np.int64))
            chunk_C.append(np.zeros((1, P), np.float32))
            continue
        e0, e1 = bound[n0], bound[n1]
        es, ed = src_s[e0:e1], dst_s[e0:e1]
        selfn = np.arange(n0, n1)
        all_s = np.concatenate([es, selfn])
        all_d = np.concatenate([ed, selfn]) - n0
        coef = np.concatenate([dinv[es] * dinv[ed], dinv[selfn] ** 2])
        uniq, inv = np.unique(all_s, return_inverse=True)
        C = np.zeros((len(uniq), P), np.float32)
        np.add.at(C, (inv, all_d), coef)
        chunk_src.append(uniq)
        chunk_C.append(C)

    counts = np.array([len(s) for s in chunk_src])
    rank = np.argsort(-counts, kind="stable")
    a_k = np.zeros(NCHUNK, np.int64)   # chunk -> core
    s_k = np.zeros(NCHUNK, np.int64)   # chunk -> slot
    T1 = []
    for s in range(CPC):
        grp = rank[s * NCORES:(s + 1) * NCORES]
        a_k[grp] = np.arange(NCORES)
        s_k[grp] = s
        T1.append(int(np.ceil(counts[grp].max() / P)))
    T1 = tuple(T1)
    ST1 = sum(T1)
    off1 = np.concatenate([[0], np.cumsum(T1)])

    # node -> h gather row: chunk k=(n//128) at (core a, slot s), partition
    # p=n%128; h tiles are [128, 10, 256] per rank -> row a*1280 + p*10 + s.
    kk = nn // P
    pp_ = nn % P
    hrow = a_k[kk] * NPC + pp_ * CPC + s_k[kk]

    gidx1 = np.zeros((NCORES, P, ST1 * 8), np.int16)
    cmat1 = np.zeros((NCORES, P, ST1, P), F8)
    for c in range(NCHUNK):
        cr, sl = a_k[c], s_k[c]
        E_s = T1[sl] * P
        idxs = np.zeros(E_s, np.int64)
        idxs[:counts[c]] = chunk_src[c]
        gidx1[cr, :, off1[sl] * 8:off1[sl + 1] * 8] = _wrap16(idxs, E_s)
        Cp = np.zeros((E_s, P), np.float32)
        Cp[:counts[c]] = chunk_C[c]
        # edge-slot e -> [partition e%128, tile e//128]
        cmat1[cr, :, off1[sl]:off1[sl + 1], :] = \
            Cp.reshape(T1[sl], P, P).transpose(1, 0, 2).astype(F8)

    # ---------------- L2: per-row-tile (labeled dst only), single phase ------
    bins = label.reshape(NCORES, RT, P)   # core c, tile r, row p -> label node
    t2 = np.zeros((NCORES, RT), np.int64)
    binsrc = {}
    for c in range(NCORES):
        for r in range(RT):
            labs = bins[c, r]
            segs, segd, segc = [], [], []
            for p in range(P):
                v = labs[p]
                e0, e1 = bound[v], bound[v + 1]
                es = src_s[e0:e1]
                segs.append(np.concatenate([es, [v]]))
                segd.append(np.full(len(es) + 1, p, np.int64))
                segc.append(np.concatenate([dinv[es] * dinv[v], [dinv[v] ** 2]]))
            all_s = np.concatenate(segs)
            all_d = np.concatenate(segd)
            coef = np.concatenate(segc)
            hr = hrow[all_s]
            uniq, inv = np.unique(hr, return_inverse=True)
            C = np.zeros((len(uniq), P), np.float32)
            np.add.at(C, (inv, all_d), coef)
            t2[c, r] = int(np.ceil(len(uniq) / P))
            binsrc[(c, r)] = (uniq, C)
    T2 = tuple(int(t2[:, r].max()) for r in range(RT))
    ST2 = sum(T2)
    off2 = np.concatenate([[0], np.cumsum(T2)])

    gidx2 = np.zeros((NCORES, P, ST2 * 8), np.int16)
    cmat2 = np.zeros((NCORES, P, ST2, P), F8)
    for c in range(NCORES):
        for r in range(RT):
            uniq, C = binsrc[(c, r)]
            Ea = T2[r] * P
            ia = np.zeros(Ea, np.int64)
            ia[:len(uniq)] = uniq
            gidx2[c, :, off2[r] * 8:off2[r + 1] * 8] = _wrap16(ia, Ea)
            Ca = np.zeros((Ea, P), np.float32)
            Ca[:len(uniq)] = C
            cmat2[c, :, off2[r]:off2[r + 1], :] = \
                Ca.reshape(T2[r], P, P).transpose(1, 0, 2).astype(F8)

    # ---------------- softmax-stability bound (cheap fp32 host forward) ------
    def _agg_all(xw):
        # fast segment sum via reduceat on the dst-sorted edges
        msg = (dinv[src_s] * dinv[dst_s])[:, None] * xw[src_s]
        agg = np.zeros_like(xw)
        nz = np.flatnonzero(np.diff(np.append(-1, dst_s)))
        agg[dst_s[nz]] = np.add.reduceat(msg, nz, axis=0)
        return agg + (dinv * dinv)[:, None] * xw

    h_np = np.maximum(_agg_all(x @ np.asarray(inputs["W_g1"], np.float32))
                      + np.asarray(inputs["b_g1"], np.float32), 0.0)
    g_np = _agg_all(h_np @ np.asarray(inputs["W_g2"], np.float32)) \
        + np.asarray(inputs["b_g2"], np.float32)
    img_np = np.maximum(image @ np.asarray(inputs["W_img1"], np.float32)
                        + np.asarray(inputs["b_img1"], np.float32), 0.0)
    img_np = np.maximum(img_np @ np.asarray(inputs["W_img2"], np.float32)
                        + np.asarray(inputs["b_img2"], np.float32), 0.0)
    bnd_logit = float(np.linalg.norm(g_np[label], axis=1).max()
                      * np.linalg.norm(img_np, axis=1).max())
    stable = bnd_logit > 60.0
    b2nz = bool(np.any(np.asarray(inputs["b_g2"], np.float32)))

    # ---------------- tensors ------------------------------------------------
    xpad = np.zeros((NPAD, D), np.float32)
    xpad[:N_NODES] = x
    xrow = np.ascontiguousarray(xpad).astype(F8)

    def km(w, kt):  # [K, M] -> [128p, kt, M]
        return np.ascontiguousarray(
            w.reshape(kt, P, w.shape[1]).transpose(1, 0, 2)
        ).astype(BF16)

    shared = {
        "xrow": xrow,
        "wg1": km(np.asarray(inputs["W_g1"], np.float32), 4),       # [128, 4, 256]
        "wg2k": np.ascontiguousarray(
            np.asarray(inputs["W_g2"], np.float32).reshape(2, P, 4, P).transpose(3, 2, 0, 1)
        ).astype(BF16),                                             # [128d, 4dblk, 2k, 128h]
        "wi1": np.ascontiguousarray(
            np.asarray(inputs["W_img1"], np.float32).reshape(4, P, 2, P).transpose(1, 0, 2, 3)
        ).astype(BF16),
        "wi2": np.ascontiguousarray(
            np.asarray(inputs["W_img2"], np.float32).reshape(2, P, 4, P).transpose(1, 0, 2, 3)
        ).astype(BF16),
        "bg1": np.asarray(inputs["b_g1"], np.float32).astype(BF16).reshape(1, Hdim),
        "bg2": np.asarray(inputs["b_g2"], np.float32).astype(BF16).reshape(1, D),
        "bi1": np.ascontiguousarray(np.asarray(inputs["b_img1"], np.float32).reshape(2, P).T),
        "bi2": np.ascontiguousarray(np.asarray(inputs["b_img2"], np.float32).reshape(4, P).T),
    }

    imageb = image.astype(np.float32)
    percore = []
    for c in range(NCORES):
        # all 4096 images, rolled so this core's own 512 come first (column
        # order of the logits is irrelevant to the row-wise LSE)
        rolled = np.concatenate([imageb[c * 512:], imageb[:c * 512]], axis=0)
        imt = np.ascontiguousarray(
            rolled.T.reshape(4, P, BATCH).transpose(1, 0, 2)
        ).astype(F8)  # [128 kpart, 4 kblk, 4096 imgs]
        labf = np.ascontiguousarray(
            label[c * 512:(c + 1) * 512].astype(np.float32).reshape(RT, P).T
        )  # [128, RT]
        percore.append({
            "cmat1": np.ascontiguousarray(cmat1[c]),
            "gidx1": np.ascontiguousarray(gidx1[c]),
            "cmat2": np.ascontiguousarray(cmat2[c]),
            "gidx2": np.ascontiguousarray(gidx2[c]),
            "imt": imt, "labf": labf,
        })
    shared["b2c"] = np.ascontiguousarray(
        np.asarray(inputs["b_g2"], np.float32).reshape(4, P).T)   # [128, 4]
    return shared, percore, (T1, T2, stable, b2nz)


def _build(key):
    """Build the SPMD Bass program."""
    T1, T2, stable, b2nz = key
    import concourse.bass as bass  # noqa: F401
    import concourse.tile as tile
    from concourse import bacc, mybir
    from concourse.masks import make_identity

    fp32 = mybir.dt.float32
    bf16 = mybir.dt.bfloat16
    f8 = mybir.dt.float8e4
    i16 = mybir.dt.int16
    AF = mybir.ActivationFunctionType
    AX = mybir.AxisListType
    DR = mybir.MatmulPerfMode.DoubleRow
    ST1 = sum(T1)
    ST2 = sum(T2)
    o1 = [0]
    for t in T1:
        o1.append(o1[-1] + t)
    off2 = [0]
    for t in T2:
        off2.append(off2[-1] + t)

    nc = bacc.Bacc("TRN2", target_bir_lowering=False, debug=False,
                   num_devices=NCORES, num_swdge_queues=4)

    t_xrow = nc.dram_tensor("xrow", [NPAD, D], f8, kind="ExternalInput").ap()
    t_wg1 = nc.dram_tensor("wg1", [P, 4, Hdim], bf16, kind="ExternalInput").ap()
    t_wg2k = nc.dram_tensor("wg2k", [P, 4, 2, P], bf16, kind="ExternalInput").ap()
    t_b2c = nc.dram_tensor("b2c", [P, 4], fp32, kind="ExternalInput").ap()
    t_wi1 = nc.dram_tensor("wi1", [P, 4, 2, P], bf16, kind="ExternalInput").ap()
    t_wi2 = nc.dram_tensor("wi2", [P, 2, 4, P], bf16, kind="ExternalInput").ap()
    t_bg1 = nc.dram_tensor("bg1", [1, Hdim], bf16, kind="ExternalInput").ap()
    t_bg2 = nc.dram_tensor("bg2", [1, D], bf16, kind="ExternalInput").ap()
    t_bi1 = nc.dram_tensor("bi1", [P, 2], fp32, kind="ExternalInput").ap()
    t_bi2 = nc.dram_tensor("bi2", [P, 4], fp32, kind="ExternalInput").ap()
    t_cmat1 = nc.dram_tensor("cmat1", [P, ST1, P], f8, kind="ExternalInput").ap()
    t_gidx1 = nc.dram_tensor("gidx1", [P, ST1 * 8], i16, kind="ExternalInput").ap()
    t_cmat2 = nc.dram_tensor("cmat2", [P, ST2, P], f8, kind="ExternalInput").ap()
    t_gidx2 = nc.dram_tensor("gidx2", [P, ST2 * 8], i16, kind="ExternalInput").ap()
    t_imt = nc.dram_tensor("imt", [P, 4, BATCH], f8, kind="ExternalInput").ap()
    t_labf = nc.dram_tensor("labf", [P, RT], fp32, kind="ExternalInput").ap()
    t_out = nc.dram_tensor("partial", [1, 1], fp32, kind="ExternalOutput").ap()

    rg = [list(range(NCORES))]

    with tile.TileContext(nc) as tc:
        from contextlib import ExitStack
        with ExitStack() as ctx:
            dram = ctx.enter_context(tc.tile_pool(name="dram", bufs=1, space="DRAM"))
            const = ctx.enter_context(tc.tile_pool(name="const", bufs=1))
            big = ctx.enter_context(tc.tile_pool(name="big", bufs=1))
            work = ctx.enter_context(tc.tile_pool(name="work", bufs=3))
            stat = ctx.enter_context(tc.tile_pool(name="stat", bufs=4))

            h_own = dram.tile([P, CPC, Hdim], f8)               # h all 10 slots
            h_t = dram.tile([NPAD, Hdim], f8, addr_space="Shared")

            # ---- constants in SBUF: gather idx first on the sync queue ----
            gidx1_s = const.tile([P, ST1 * 8], i16)
            nc.sync.dma_start(out=gidx1_s[:], in_=t_gidx1[:])
            cm1_s = const.tile([P, ST1, P], f8)
            nc.scalar.dma_start(out=cm1_s[:], in_=t_cmat1[:])
            wi1_s = const.tile([P, 4, 2, P], bf16)
            nc.sync.dma_start(out=wi1_s[:], in_=t_wi1[:])
            wi2_s = const.tile([P, 2, 4, P], bf16)
            nc.sync.dma_start(out=wi2_s[:], in_=t_wi2[:])
            bi1_s = const.tile([P, 2], fp32)
            nc.sync.dma_start(out=bi1_s[:], in_=t_bi1[:])
            bi2_s = const.tile([P, 4], fp32)
            nc.sync.dma_start(out=bi2_s[:], in_=t_bi2[:])
            wg1_s = const.tile([P, 4, Hdim], bf16)
            nc.scalar.dma_start(out=wg1_s[:], in_=t_wg1[:])
            gidx2_s = const.tile([P, ST2 * 8], i16)
            nc.scalar.dma_start(out=gidx2_s[:], in_=t_gidx2[:])
            cm2_s = const.tile([P, ST2, P], f8)
            nc.scalar.dma_start(out=cm2_s[:], in_=t_cmat2[:])
            bg1_s = const.tile([1, Hdim], bf16)
            nc.scalar.dma_start(out=bg1_s[:], in_=t_bg1[:])
            bg2_s = const.tile([1, D], bf16)
            nc.scalar.dma_start(out=bg2_s[:], in_=t_bg2[:])
            wg2_s = const.tile([P, 4, 2, P], bf16)
            nc.scalar.dma_start(out=wg2_s[:], in_=t_wg2k[:])
            b2c_s = const.tile([P, 4], fp32)
            nc.scalar.dma_start(out=b2c_s[:], in_=t_b2c[:])
            labf_s = const.tile([P, RT], fp32)
            nc.scalar.dma_start(out=labf_s[:], in_=t_labf[:])
            imt_s = const.tile([P, 4, BATCH], f8)
            nc.scalar.dma_start(out=imt_s[:], in_=t_imt[:])
            ones_row = const.tile([1, P], bf16)
            nc.vector.memset(ones_row[:], 1.0)
            ones_cb = const.tile([P, 1], bf16)
            nc.vector.memset(ones_cb[:], 1.0)
            ones_col = const.tile([P, 1], fp32)
            nc.vector.memset(ones_col[:], 1.0)
            ident_b = const.tile([P, P], bf16)
            make_identity(nc, ident_b[:])

            # warm-up: ramp the PE p-state and preload scalar activation
            # tables while the input DMAs stream (otherwise the first L1 slot
            # chain pays the cold-clock + table-load penalty)
            warm = const.tile([P, P], bf16)
            with tc.tile_pool(name="ps_warm", bufs=1, space="PSUM") as ps_warm:
                pw = ps_warm.tile([P, P], fp32)
                for _ in range(12):
                    nc.tensor.matmul(out=pw[:], lhsT=ident_b[:], rhs=ident_b[:],
                                     start=True, stop=True)
                wa = const.tile([P, 16], fp32)
                nc.vector.tensor_copy(out=warm[:, 0:16], in_=pw[:, 0:16])
                nc.scalar.activation(out=wa[:], in_=pw[:, 0:16], func=AF.Relu)
                nc.scalar.activation(out=wa[:], in_=pw[:, 0:16], func=AF.Exp)
                nc.scalar.activation(out=wa[:], in_=pw[:, 0:16], func=AF.Ln)

            imgT8 = big.tile([P, 4, BATCH], f8)     # ALL image encodings (local MLP)
            M_s = big.tile([P, 2, BATCH], f8)       # M = W_g2^T-fused image side
            diag_s = stat.tile([P, RT], fp32)
            contrib = stat.tile([P, RT], fp32)

            # ===== image MLP, replicated over ALL 4096 images =================
            # n-tile 0 (this core's own images) runs first; tiles 1-7 are
            # emitted AFTER the h AllGather trigger so TensorE fills the
            # otherwise-idle mesh-wait window
            h1t = big.tile([P, 2, BATCH], f8)

            def mlp_tile(n, pool, tag1, tag2):
                sl = slice(n * 512, (n + 1) * 512)
                for m in range(2):
                    pm = pool.tile([P, 512], fp32, tag=tag1)
                    for k in range(4):
                        nc.tensor.matmul(
                            out=pm[:], lhsT=wi1_s[:, k, m, :], rhs=imt_s[:, k, sl],
                            start=(k == 0), stop=(k == 3),
                        )
                    nc.scalar.activation(
                        out=h1t[:, m, sl], in_=pm[:], func=AF.Relu,
                        bias=bi1_s[:, m:m + 1], scale=1.0,
                    )
                for m in range(4):
                    pm2 = pool.tile([P, 512], fp32, tag=tag2)
                    for k in range(2):
                        nc.tensor.matmul(
                            out=pm2[:], lhsT=wi2_s[:, k, m, :], rhs=h1t[:, k, sl],
                            start=(k == 0), stop=(k == 1),
                        )
                    nc.scalar.activation(
                        out=imgT8[:, m, sl], in_=pm2[:], func=AF.Relu,
                        bias=bi2_s[:, m:m + 1], scale=1.0,
                    )

            # ===== GCN layer 1: all slot gathers issued up front ==============
            # two pieces per slot (<=9 tiles each fits the SWDGE ring), pieces
            # round-robined over the 4 queues so all generators stay fed; the
            # image AllGather is slipped in after round 2 (the engine would be
            # blocked on queue backpressure then anyway)
            ghs = []          # per slot: two piece tiles (piece-granular deps)
            pieces = []
            for s in range(CPC):
                half = (T1[s] + 1) // 2
                gha = big.tile([P, half, D], f8, name=f"gh{s}a")
                ghb = big.tile([P, T1[s] - half, D], f8, name=f"gh{s}b")
                pieces.append((s, 0, half, gha))
                pieces.append((s, half, T1[s], ghb))
                ghs.append((gha, ghb, half))
            for qn, (s, a, b, gt) in enumerate(pieces):  # noqa: B007
                nc.gpsimd.dma_gather(
                    out_ap=gt[:, :, :], in_ap=t_xrow[:, :],
                    idxs_ap=gidx1_s[:, (o1[s] + a) * 8:(o1[s] + b) * 8],
                    num_idxs=(b - a) * P, num_idxs_reg=(b - a) * P,
                    elem_size=D, single_packet=False,
                    queue_num=qn % 4,
                )

            with tc.tile_pool(name="ps_l1", bufs=2, space="PSUM") as ps_l1:
                def l1_slot(s):
                    pa = ps_l1.tile([P, D], fp32, tag="agg1", name="pa")
                    gha, ghb, half = ghs[s]
                    for (gt, a, b) in ((gha, 0, half), (ghb, half, T1[s])):
                        j = a
                        while j < b:
                            if j + 1 < b:
                                nc.tensor.matmul(
                                    out=pa[:], lhsT=cm1_s[:, o1[s] + j:o1[s] + j + 2, :],
                                    rhs=gt[:, j - a:j - a + 2, :],
                                    start=(j == 0), stop=(j + 2 == T1[s]), perf_mode=DR,
                                )
                                j += 2
                            else:
                                nc.tensor.matmul(
                                    out=pa[:], lhsT=cm1_s[:, o1[s] + j, :],
                                    rhs=gt[:, j - a, :],
                                    start=(j == 0), stop=(j + 1 == T1[s]),
                                )
                                j += 1
                    a1 = work.tile([P, D], bf16, tag="a1", name="a1")
                    nc.vector.tensor_copy(out=a1[:], in_=pa[:])
                    a1t = work.tile([P, 4, P], bf16, tag="a1t", name="a1t")
                    for k in range(4):
                        pt1 = ps_l1.tile([P, P], bf16, tag="tps1", name="pt1")
                        nc.tensor.transpose(
                            out=pt1[:], in_=a1[:, k * P:(k + 1) * P], identity=ident_b[:]
                        )
                        nc.vector.tensor_copy(out=a1t[:, k, :], in_=pt1[:])
                    ph = ps_l1.tile([P, Hdim], fp32, tag="hps", name="ph")
                    for k in range(4):
                        nc.tensor.matmul(
                            out=ph[:], lhsT=a1t[:, k, :], rhs=wg1_s[:, k, :],
                            start=(k == 0), stop=False,
                        )
                    nc.tensor.matmul(
                        out=ph[:], lhsT=ones_row[:], rhs=bg1_s[:],
                        start=False, stop=True, skip_group_check=True,
                    )
                    h_sb = work.tile([P, Hdim], f8, tag="h_sb", name="h_sb")
                    nc.scalar.activation(out=h_sb[:], in_=ph[:], func=AF.Relu)
                    nc.sync.dma_start(out=h_own[:, s, :], in_=h_sb[:])

                for s in range(CPC):
                    l1_slot(s)
                nc.gpsimd.collective_compute(
                    "AllGather", mybir.AluOpType.bypass, replica_groups=rg,
                    ins=[h_own[:, :, :]], outs=[h_t[:, :]],
                )
                for n in range(NT):
                    mlp_tile(n, ps_l1, "agg1", "agg1")
                # M[h, j] = sum_d W_g2[h? -- see layout] : fuse W_g2 into the
                # image side so logits contract over Hdim=256 instead of D=512
                if b2nz:
                    bterm_s = big.tile([1, BATCH], fp32)
                for n in range(NT):
                    sl = slice(n * 512, (n + 1) * 512)
                    for k in range(2):
                        pmm = ps_l1.tile([P, 512], fp32, tag="agg1")
                        for d in range(4):
                            nc.tensor.matmul(
                                out=pmm[:], lhsT=wg2_s[:, d, k, :],
                                rhs=imgT8[:, d, sl],
                                start=(d == 0), stop=(d == 3),
                            )
                        nc.vector.tensor_copy(out=M_s[:, k, sl], in_=pmm[:])
                    if b2nz:
                        pbt = ps_l1.tile([1, 512], fp32, tag="hps")
                        for d in range(4):
                            nc.tensor.matmul(
                                out=pbt[:], lhsT=b2c_s[:, d:d + 1],
                                rhs=imgT8[:, d, sl],
                                start=(d == 0), stop=(d == 3),
                            )
                        nc.vector.tensor_copy(out=bterm_s[:, sl], in_=pbt[:])

            # ===== GCN layer 2 + logits/LSE ===================================
            T2m = max(T2)
            with ExitStack() as c2:
                ps_l2 = c2.enter_context(
                    tc.tile_pool(name="ps_l2", bufs=1, space="PSUM"))
                ps_lg = c2.enter_context(
                    tc.tile_pool(name="ps_lg", bufs=4, space="PSUM"))

                # gathers (2 pieces per row tile, round-robin queues,
                # piece-granular tiles so r0's MMs start after round 1)
                ga_t = []
                qn = 0
                for r in range(RT):
                    half = (T2[r] + 1) // 2
                    gaa = big.tile([P, half, Hdim], f8, name=f"ga{r}a")
                    gab = big.tile([P, T2[r] - half, Hdim], f8, name=f"ga{r}b")
                    for (a, b, gt) in ((0, half, gaa), (half, T2[r], gab)):
                        nc.gpsimd.dma_gather(
                            out_ap=gt[:, :, :], in_ap=h_t[:, :],
                            idxs_ap=gidx2_s[:, (off2[r] + a) * 8:(off2[r] + b) * 8],
                            num_idxs=(b - a) * P, num_idxs_reg=(b - a) * P,
                            elem_size=Hdim, single_packet=False,
                            queue_num=qn % 4,
                        )
                        qn += 1
                    ga_t.append((gaa, gab, half))

                # per row tile: aggregation, txtT, diag, logits + LSE
                for r in range(RT):
                    pa2 = ps_l2.tile([P, Hdim], fp32, tag="agg2", name="pa2", bufs=2)
                    gaa, gab, half = ga_t[r]
                    for j in range(T2[r]):
                        gt, jj = (gaa, j) if j < half else (gab, j - half)
                        nc.tensor.matmul(
                            out=pa2[:], lhsT=cm2_s[:, off2[r] + j, :],
                            rhs=gt[:, jj, :],
                            start=(j == 0), stop=(j + 1 == T2[r]),
                        )
                    a2 = work.tile([P, Hdim], bf16, tag="a2")
                    nc.vector.tensor_copy(out=a2[:], in_=pa2[:])
                    a2t8 = work.tile([P, 2, P], f8, tag="a2t")
                    for k in range(2):
                        pt = ps_l2.tile([P, P], bf16, tag="tps")
                        nc.tensor.transpose(
                            out=pt[:], in_=a2[:, k * P:(k + 1) * P], identity=ident_b[:]
                        )
                        nc.vector.tensor_copy(out=a2t8[:, k, :], in_=pt[:])
                    # diag = diagonal of (a2t8^T @ M_own): identity-mask + row sum
                    px = ps_l2.tile([P, P], fp32, tag="ptx")
                    for k in range(2):
                        nc.tensor.matmul(
                            out=px[:], lhsT=a2t8[:, k, :],
                            rhs=M_s[:, k, r * P:(r + 1) * P],
                            start=(k == 0), stop=(k == 1),
                        )
                    xs = work.tile([P, P], bf16, tag="xs")
                    nc.vector.tensor_copy(out=xs[:], in_=px[:])
                    xm = work.tile([P, P], bf16, tag="xm")
                    nc.vector.tensor_tensor(
                        out=xm[:], in0=xs[:], in1=ident_b[:],
                        op=mybir.AluOpType.mult,
                    )
                    nc.vector.reduce_sum(out=diag_s[:, r:r + 1], in_=xm[:], axis=AX.X)

                    # logits row-tile x full imgT, fused exp+accumulate
                    sums = stat.tile([P, NT], fp32, tag="sums")
                    if stable:
                        banks = []
                    for n in range(NT):
                        pl = ps_lg.tile([P, 512], fp32, tag="lg")
                        nc.tensor.matmul(
                            out=pl[:], lhsT=a2t8[:, :, :],
                            rhs=M_s[:, :, n * 512:(n + 1) * 512],
                            start=True, stop=not b2nz, perf_mode=DR,
                        )
                        if b2nz:
                            nc.tensor.matmul(
                                out=pl[:], lhsT=ones_row[:],
                                rhs=bterm_s[:, n * 512:(n + 1) * 512],
                                start=False, stop=True, skip_group_check=True,
                            )
                        if stable:
                            banks.append(pl)
                        else:
                            esc = work.tile([P, 512], bf16, tag="esc", bufs=1)
                            nc.scalar.activation(
                                out=esc[:], in_=pl[:], func=AF.Exp,
                                accum_out=sums[:, n:n + 1],
                            )
                    if stable:
                        maxes = stat.tile([P, NT], fp32, tag="maxes")
                        for n in range(NT):
                            nc.vector.reduce_max(out=maxes[:, n:n + 1], in_=banks[n][:], axis=AX.X)
                        rmax = stat.tile([P, 1], fp32, tag="rmax")
                        nc.vector.reduce_max(out=rmax[:], in_=maxes[:], axis=AX.X)
                        nrmax = stat.tile([P, 1], fp32, tag="nrmax")
                        nc.scalar.mul(nrmax[:], rmax[:], -1.0)
                        for n in range(NT):
                            esc = work.tile([P, 512], bf16, tag="esc")
                            nc.scalar.activation(
                                out=esc[:], in_=banks[n][:], func=AF.Exp,
                                bias=nrmax[:], scale=1.0, accum_out=sums[:, n:n + 1],
                            )
                    ssum = stat.tile([P, 1], fp32, tag="ssum")
                    nc.vector.reduce_sum(out=ssum[:], in_=sums[:], axis=AX.X)
                    lns = stat.tile([P, 1], fp32, tag="lns")
                    nc.scalar.activation(out=lns[:], in_=ssum[:], func=AF.Ln)
                    t1 = stat.tile([P, 1], fp32, tag="t1")
                    if stable:
                        nc.vector.tensor_add(out=t1[:], in0=rmax[:], in1=lns[:])
                        nc.vector.tensor_sub(out=t1[:], in0=t1[:], in1=diag_s[:, r:r + 1])
                    else:
                        nc.vector.tensor_sub(out=t1[:], in0=lns[:], in1=diag_s[:, r:r + 1])
                    nc.vector.tensor_mul(
                        out=contrib[:, r:r + 1], in0=t1[:], in1=labf_s[:, r:r + 1]
                    )

            rsum = stat.tile([P, 1], fp32, tag="rsum")
            nc.vector.reduce_sum(out=rsum[:], in_=contrib[:], axis=AX.X)
            with tc.tile_pool(name="ps_fin", bufs=1, space="PSUM") as ps_fin:
                pf = ps_fin.tile([1, 1], fp32)
                nc.tensor.matmul(out=pf[:], lhsT=rsum[:], rhs=ones_col[:], start=True, stop=True)
                fin = stat.tile([1, 1], fp32, tag="fin")
                nc.vector.tensor_copy(out=fin[:], in_=pf[:])
            nc.sync.dma_start(out=t_out[:], in_=fin[:])

    nc.compile()
    return nc


_CACHE = {}


def kernel(**inputs) -> np.ndarray:
    from concourse.bass_utils import run_bass_kernel_spmd

    shared, percore, key = _prep(inputs)
    ckey = (key[0], key[1], key[2], key[3])
    if ckey not in _CACHE:
        _CACHE[ckey] = _build(ckey)
    nc = _CACHE[ckey]

    in_maps = []
    for c in range(NCORES):
        m = dict(shared)
        pc = percore[c]
        m.update({"cmat1": pc["cmat1"], "gidx1": pc["gidx1"],
                  "cmat2": pc["cmat2"], "gidx2": pc["gidx2"],
                  "imt": pc["imt"], "labf": pc["labf"]})
        in_maps.append(m)

    trace = bool(int(os.environ.get("KERNEL_TRACE", "0")))
    try:
        res = run_bass_kernel_spmd(nc, in_maps, core_ids=list(range(NCORES)), trace=trace)
    except Exception:
        # transient NRT/device hiccups have been observed to clear on retry
        res = run_bass_kernel_spmd(nc, in_maps, core_ids=list(range(NCORES)), trace=trace)
    kernel.last_results = res
    total = sum(float(r["partial"][0, 0]) for r in res.results)
    return np.float32(total / BATCH + 1.0)


# revision 36
# speedup vs baseline: 1.0399x; 1.0399x over previous
"""Trainium2 Bass kernel for nn_CLIP_GCN_Model (2-layer GCN + MLP + contrastive loss).

Reformulation (validated numerically):
  out = mean_i(label_i * (lse_i - logits_ii)) + 1.0
(the triplet term of the reference is identically 1.0).

GCN layer: out = S @ (x @ W) + b where S = D^-1/2 (A+I) D^-1/2.

Structure (single collective, eager gather streams):
  1. L1 runs over all 10240 (padded) nodes: 80 dst-chunks of 128 nodes,
     balanced to (core, slot); per chunk the distinct source x rows (512B fp8)
     are dma_gathered and aggregated with a fp8 coefficient matrix C
     (DoubleRow matmuls in PSUM), then transposed, x W_g1 + bias + relu -> h.
     All 20 gather pieces (2 per slot, <=9 tiles each to fit the SWDGE ring)
     are dispatched up front, round-robined over the 4 queues so all four
     descriptor generators run concurrently.
  2. h slots are written p-major ([128, 10, 256] per rank; node row =
     rank*1280 + p*10 + slot) and shared with ONE AllGather -> h_t.
  3. The image MLP is fully replicated: every core encodes ALL 4096 images
     (fp8, column-rolled so its own 512 images sit in block 0 -- the row-wise
     LSE is invariant to logits column order). It is emitted AFTER the h
     AllGather trigger so TensorE fills the otherwise-idle mesh-wait window,
     and it removes the image AllGather entirely. A small warm-up block ramps
     the PE p-state / scalar activation tables before the first L1 slot.
  4. W_g2 is fused into the image side: M = W_g2-contracted image encodings
     [256, 4096] is computed once per core in the AllGather window, so each
     128-label row tile only needs its h aggregation [128, 256], a transpose,
     and ONE fp8 DoubleRow matmul per 512-column tile (K=256 instead of 512).
     The diagonal is extracted as diag(a2t^T @ M_own) via identity mask +
     row reduce. LSE uses fused exp+accumulate (fast path: a host-side bound
     check shows exp cannot overflow in this data regime).
"""

import os
import numpy as np
import ml_dtypes

BF16 = ml_dtypes.bfloat16
F8 = ml_dtypes.float8_e4m3   # TRN fp8e4 (max 240)

N_NODES = 10000
NPAD = 10240
D = 512
Hdim = 256
BATCH = 4096
NCORES = 8
P = 128
NCHUNK = NPAD // P          # 80
CPC = NCHUNK // NCORES      # 10 slots per core
NPC = NPAD // NCORES        # 1280 nodes per core
MPC = NPC // P              # 10 m-tiles per core
RT = 4                      # row tiles per core (512 rows each core)
NT = BATCH // 512           # 8 column tiles of 512
H5 = CPC // 2               # 5 slots per h-half
HALF_N = NPAD // 2          # 5120


def _wrap16(idx, n):
    """Layout indices for dma_gather: element i -> [i%16, i//16], replicated to 128 partitions."""
    assert len(idx) == n and n % 16 == 0
    base = idx.astype(np.int16).reshape(n // 16, 16).T  # [16, n/16]
    return np.ascontiguousarray(np.tile(base, (8, 1)))  # [128, n/16]


def _prep(inputs):
    """Host-side layout/sharding prep."""
    x = np.asarray(inputs["x_nodes"], dtype=np.float32)
    image = np.asarray(inputs["image"], dtype=np.float32)
    ei = np.asarray(inputs["edge_index"]).astype(np.int64)
    label = np.asarray(inputs["label"]).astype(np.int64)
    src, dst = ei[0], ei[1]

    deg = np.ones(N_NODES, np.float32)
    np.add.at(deg, dst, 1.0)
    dinv = (1.0 / np.sqrt(deg)).astype(np.float32)

    # in-edges grouped by dst (sorted once)
    order = np.argsort(dst, kind="stable")
    src_s, dst_s = src[order], dst[order]
    bound = np.searchsorted(dst_s, np.arange(N_NODES + 1))

    nn = np.arange(NPAD)

    # ---------------- L1: per-chunk dedup + balanced (core,slot) assignment --
    chunk_src = []      # distinct sources per chunk
    chunk_C = []        # [n_distinct, 128] fp32 coef
    for c in range(NCHUNK):
        n0, n1 = c * P, min((c + 1) * P, N_NODES)
        if n0 >= N_NODES:
            chunk_src.append(np.zeros(1, np.int64))
            chunk_C.append(np.zeros((1, P), np.float32))
            continue
        e0, e1 = bound[n0], bound[n1]
        es, ed = src_s[e0:e1], dst_s[e0:e1]
        selfn = np.arange(n0, n1)
        all_s = np.concatenate([es, selfn])
        all_d = np.concatenate([ed, selfn]) - n0
        coef = np.concatenate([dinv[es] * dinv[ed], dinv[selfn] ** 2])
        uniq, inv = np.unique(all_s, return_inverse=True)
        C = np.zeros((len(uniq), P), np.float32)
        np.add.at(C, (inv, all_d), coef)
        chunk_src.append(uniq)
        chunk_C.append(C)

    counts = np.array([len(s) for s in chunk_src])
    rank = np.argsort(-counts, kind="stable")
    a_k = np.zeros(NCHUNK, np.int64)   # chunk -> core
    s_k = np.zeros(NCHUNK, np.int64)   # chunk -> slot
    T1 = []
    for s in range(CPC):
        grp = rank[s * NCORES:(s + 1) * NCORES]
        a_k[grp] = np.arange(NCORES)
        s_k[grp] = s
        T1.append(int(np.ceil(counts[grp].max() / P)))
    T1 = tuple(T1)
    ST1 = sum(T1)
    off1 = np.concatenate([[0], np.cumsum(T1)])

    # node -> h gather row: chunk k=(n//128) at (core a, slot s), partition
    # p=n%128; h tiles are [128, 10, 256] per rank -> row a*1280 + p*10 + s.
    kk = nn // P
    pp_ = nn % P
    hrow = a_k[kk] * NPC + pp_ * CPC + s_k[kk]

    gidx1 = np.zeros((NCORES, P, ST1 * 8), np.int16)
    cmat1 = np.zeros((NCORES, P, ST1, P), F8)
    for c in range(NCHUNK):
        cr, sl = a_k[c], s_k[c]
        E_s = T1[sl] * P
        idxs = np.zeros(E_s, np.int64)
        idxs[:counts[c]] = chunk_src[c]
        gidx1[cr, :, off1[sl] * 8:off1[sl + 1] * 8] = _wrap16(idxs, E_s)
        Cp = np.zeros((E_s, P), np.float32)
        Cp[:counts[c]] = chunk_C[c]
        # edge-slot e -> [partition e%128, tile e//128]
        cmat1[cr, :, off1[sl]:off1[sl + 1], :] = \
            Cp.reshape(T1[sl], P, P).transpose(1, 0, 2).astype(F8)

    # ---------------- L2: per-row-tile (labeled dst only), single phase ------
    bins = label.reshape(NCORES, RT, P)   # core c, tile r, row p -> label node
    t2 = np.zeros((NCORES, RT), np.int64)
    binsrc = {}
    for c in range(NCORES):
        for r in range(RT):
            labs = bins[c, r]
            segs, segd, segc = [], [], []
            for p in range(P):
                v = labs[p]
                e0, e1 = bound[v], bound[v + 1]
                es = src_s[e0:e1]
                segs.append(np.concatenate([es, [v]]))
                segd.append(np.full(len(es) + 1, p, np.int64))
                segc.append(np.concatenate([dinv[es] * dinv[v], [dinv[v] ** 2]]))
            all_s = np.concatenate(segs)
            all_d = np.concatenate(segd)
            coef = np.concatenate(segc)
            hr = hrow[all_s]
            uniq, inv = np.unique(hr, return_inverse=True)
            C = np.zeros((len(uniq), P), np.float32)
            np.add.at(C, (inv, all_d), coef)
            t2[c, r] = int(np.ceil(len(uniq) / P))
            binsrc[(c, r)] = (uniq, C)
    T2 = tuple(int(t2[:, r].max()) for r in range(RT))
    ST2 = sum(T2)
    off2 = np.concatenate([[0], np.cumsum(T2)])

    gidx2 = np.zeros((NCORES, P, ST2 * 8), np.int16)
    cmat2 = np.zeros((NCORES, P, ST2, P), F8)
    for c in range(NCORES):
        for r in range(RT):
            uniq, C = binsrc[(c, r)]
            Ea = T2[r] * P
            ia = np.zeros(Ea, np.int64)
            ia[:len(uniq)] = uniq
            gidx2[c, :, off2[r] * 8:off2[r + 1] * 8] = _wrap16(ia, Ea)
            Ca = np.zeros((Ea, P), np.float32)
            Ca[:len(uniq)] = C
            cmat2[c, :, off2[r]:off2[r + 1], :] = \
                Ca.reshape(T2[r], P, P).transpose(1, 0, 2).astype(F8)

    # ---------------- softmax-stability bound (cheap fp32 host forward) ------
    def _agg_all(xw):
        # fast segment sum via reduceat on the dst-sorted edges
        msg = (dinv[src_s] * dinv[dst_s])[:, None] * xw[src_s]
        agg = np.zeros_like(xw)
        nz = np.flatnonzero(np.diff(np.append(-1, dst_s)))
        agg[dst_s[nz]] = np.add.reduceat(msg, nz, axis=0)
        return agg + (dinv * dinv)[:, None] * xw

    h_np = np.maximum(_agg_all(x @ np.asarray(inputs["W_g1"], np.float32))
                      + np.asarray(inputs["b_g1"], np.float32), 0.0)
    g_np = _agg_all(h_np @ np.asarray(inputs["W_g2"], np.float32)) \
        + np.asarray(inputs["b_g2"], np.float32)
    img_np = np.maximum(image @ np.asarray(inputs["W_img1"], np.float32)
                        + np.asarray(inputs["b_img1"], np.float32), 0.0)
    img_np = np.maximum(img_np @ np.asarray(inputs["W_img2"], np.float32)
                        + np.asarray(inputs["b_img2"], np.float32), 0.0)
    bnd_logit = float(np.linalg.norm(g_np[label], axis=1).max()
                      * np.linalg.norm(img_np, axis=1).max())
    stable = bnd_logit > 60.0
    b2nz = bool(np.any(np.asarray(inputs["b_g2"], np.float32)))

    # ---------------- tensors ------------------------------------------------
    xpad = np.zeros((NPAD, D), np.float32)
    xpad[:N_NODES] = x
    xrow = np.ascontiguousarray(xpad).astype(F8)

    def km(w, kt):  # [K, M] -> [128p, kt, M]
        return np.ascontiguousarray(
            w.reshape(kt, P, w.shape[1]).transpose(1, 0, 2)
        ).astype(BF16)

    shared = {
        "xrow": xrow,
        "wg1": km(np.asarray(inputs["W_g1"], np.float32), 4),       # [128, 4, 256]
        "wg2k": np.ascontiguousarray(
            np.asarray(inputs["W_g2"], np.float32).reshape(2, P, 4, P).transpose(3, 2, 0, 1)
        ).astype(BF16),                                             # [128d, 4dblk, 2k, 128h]
        "wi1": np.ascontiguousarray(
            np.asarray(inputs["W_img1"], np.float32).reshape(4, P, 2, P).transpose(1, 0, 2, 3)
        ).astype(BF16),
        "wi2": np.ascontiguousarray(
            np.asarray(inputs["W_img2"], np.float32).reshape(2, P, 4, P).transpose(1, 0, 2, 3)
        ).astype(BF16),
        "bg1": np.asarray(inputs["b_g1"], np.float32).astype(BF16).reshape(1, Hdim),
        "bg2": np.asarray(inputs["b_g2"], np.float32).astype(BF16).reshape(1, D),
        "bi1": np.ascontiguousarray(np.asarray(inputs["b_img1"], np.float32).reshape(2, P).T),
        "bi2": np.ascontiguousarray(np.asarray(inputs["b_img2"], np.float32).reshape(4, P).T),
    }

    imageb = image.astype(np.float32)
    percore = []
    for c in range(NCORES):
        # all 4096 images, rolled so this core's own 512 come first (column
        # order of the logits is irrelevant to the row-wise LSE)
        rolled = np.concatenate([imageb[c * 512:], imageb[:c * 512]], axis=0)
        imt = np.ascontiguousarray(
            rolled.T.reshape(4, P, BATCH).transpose(1, 0, 2)
        ).astype(F8)  # [128 kpart, 4 kblk, 4096 imgs]
        labf = np.ascontiguousarray(
            label[c * 512:(c + 1) * 512].astype(np.float32).reshape(RT, P).T
        )  # [128, RT]
        percore.append({
            "cmat1": np.ascontiguousarray(cmat1[c]),
            "gidx1": np.ascontiguousarray(gidx1[c]),
            "cmat2": np.ascontiguousarray(cmat2[c]),
            "gidx2": np.ascontiguousarray(gidx2[c]),
            "imt": imt, "labf": labf,
        })
    shared["b2c"] = np.ascontiguousarray(
        np.asarray(inputs["b_g2"], np.float32).reshape(4, P).T)   # [128, 4]
    return shared, percore, (T1, T2, stable, b2nz)


def _build(key):
    """Build the SPMD Bass program."""
    T1, T2, stable, b2nz = key
    import concourse.bass as bass  # noqa: F401
    import concourse.tile as tile
    from concourse import bacc, mybir
    from concourse.masks import make_identity

    fp32 = mybir.dt.float32
    bf16 = mybir.dt.bfloat16
    f8 = mybir.dt.float8e4
    i16 = mybir.dt.int16
    AF = mybir.ActivationFunctionType
    AX = mybir.AxisListType
    DR = mybir.MatmulPerfMode.DoubleRow
    ST1 = sum(T1)
    ST2 = sum(T2)
    o1 = [0]
    for t in T1:
        o1.append(o1[-1] + t)
    off2 = [0]
    for t in T2:
        off2.append(off2[-1] + t)

    nc = bacc.Bacc("TRN2", target_bir_lowering=False, debug=False,
                   num_devices=NCORES, num_swdge_queues=4)

    t_xrow = nc.dram_tensor("xrow", [NPAD, D], f8, kind="ExternalInput").ap()
    t_wg1 = nc.dram_tensor("wg1", [P, 4, Hdim], bf16, kind="ExternalInput").ap()
    t_wg2k = nc.dram_tensor("wg2k", [P, 4, 2, P], bf16, kind="ExternalInput").ap()
    t_b2c = nc.dram_tensor("b2c", [P, 4], fp32, kind="ExternalInput").ap()
    t_wi1 = nc.dram_tensor("wi1", [P, 4, 2, P], bf16, kind="ExternalInput").ap()
    t_wi2 = nc.dram_tensor("wi2", [P, 2, 4, P], bf16, kind="ExternalInput").ap()
    t_bg1 = nc.dram_tensor("bg1", [1, Hdim], bf16, kind="ExternalInput").ap()
    t_bg2 = nc.dram_tensor("bg2", [1, D], bf16, kind="ExternalInput").ap()
    t_bi1 = nc.dram_tensor("bi1", [P, 2], fp32, kind="ExternalInput").ap()
    t_bi2 = nc.dram_tensor("bi2", [P, 4], fp32, kind="ExternalInput").ap()
    t_cmat1 = nc.dram_tensor("cmat1", [P, ST1, P], f8, kind="ExternalInput").ap()
    t_gidx1 = nc.dram_tensor("gidx1", [P, ST1 * 8], i16, kind="ExternalInput").ap()
    t_cmat2 = nc.dram_tensor("cmat2", [P, ST2, P], f8, kind="ExternalInput").ap()
    t_gidx2 = nc.dram_tensor("gidx2", [P, ST2 * 8], i16, kind="ExternalInput").ap()
    t_imt = nc.dram_tensor("imt", [P, 4, BATCH], f8, kind="ExternalInput").ap()
    t_labf = nc.dram_tensor("labf", [P, RT], fp32, kind="ExternalInput").ap()
    t_out = nc.dram_tensor("partial", [1, 1], fp32, kind="ExternalOutput").ap()

    rg = [list(range(NCORES))]

    with tile.TileContext(nc) as tc:
        from contextlib import ExitStack
        with ExitStack() as ctx:
            dram = ctx.enter_context(tc.tile_pool(name="dram", bufs=1, space="DRAM"))
            const = ctx.enter_context(tc.tile_pool(name="const", bufs=1))
            big = ctx.enter_context(tc.tile_pool(name="big", bufs=1))
            work = ctx.enter_context(tc.tile_pool(name="work", bufs=3))
            stat = ctx.enter_context(tc.tile_pool(name="stat", bufs=4))

            h_own = dram.tile([P, CPC, Hdim], f8)               # h all 10 slots
            h_t = dram.tile([NPAD, Hdim], f8, addr_space="Shared")

            # ---- constants in SBUF: gather idx first on the sync queue ----
            gidx1_s = const.tile([P, ST1 * 8], i16)
            nc.sync.dma_start(out=gidx1_s[:], in_=t_gidx1[:])
            imt_s = const.tile([P, 4, BATCH], f8)
            nc.sync.dma_start(out=imt_s[:], in_=t_imt[:])
            cm1_s = const.tile([P, ST1, P], f8)
            nc.scalar.dma_start(out=cm1_s[:], in_=t_cmat1[:])
            wi1_s = const.tile([P, 4, 2, P], bf16)
            nc.sync.dma_start(out=wi1_s[:], in_=t_wi1[:])
            wi2_s = const.tile([P, 2, 4, P], bf16)
            nc.sync.dma_start(out=wi2_s[:], in_=t_wi2[:])
            bi1_s = const.tile([P, 2], fp32)
            nc.sync.dma_start(out=bi1_s[:], in_=t_bi1[:])
            bi2_s = const.tile([P, 4], fp32)
            nc.sync.dma_start(out=bi2_s[:], in_=t_bi2[:])
            wg1_s = const.tile([P, 4, Hdim], bf16)
            nc.scalar.dma_start(out=wg1_s[:], in_=t_wg1[:])
            gidx2_s = const.tile([P, ST2 * 8], i16)
            nc.scalar.dma_start(out=gidx2_s[:], in_=t_gidx2[:])
            cm2_s = const.tile([P, ST2, P], f8)
            nc.scalar.dma_start(out=cm2_s[:], in_=t_cmat2[:])
            bg1_s = const.tile([1, Hdim], bf16)
            nc.scalar.dma_start(out=bg1_s[:], in_=t_bg1[:])
            bg2_s = const.tile([1, D], bf16)
            nc.scalar.dma_start(out=bg2_s[:], in_=t_bg2[:])
            wg2_s = const.tile([P, 4, 2, P], bf16)
            nc.scalar.dma_start(out=wg2_s[:], in_=t_wg2k[:])
            b2c_s = const.tile([P, 4], fp32)
            nc.scalar.dma_start(out=b2c_s[:], in_=t_b2c[:])
            labf_s = const.tile([P, RT], fp32)
            nc.scalar.dma_start(out=labf_s[:], in_=t_labf[:])
            ones_row = const.tile([1, P], bf16)
            nc.vector.memset(ones_row[:], 1.0)
            ones_cb = const.tile([P, 1], bf16)
            nc.vector.memset(ones_cb[:], 1.0)
            ones_col = const.tile([P, 1], fp32)
            nc.vector.memset(ones_col[:], 1.0)
            ident_b = const.tile([P, P], bf16)
            make_identity(nc, ident_b[:])

            # warm-up: ramp the PE p-state and preload scalar activation
            # tables while the input DMAs stream (otherwise the first L1 slot
            # chain pays the cold-clock + table-load penalty)
            warm = const.tile([P, P], bf16)
            with tc.tile_pool(name="ps_warm", bufs=1, space="PSUM") as ps_warm:
                pw = ps_warm.tile([P, P], fp32)
                for _ in range(12):
                    nc.tensor.matmul(out=pw[:], lhsT=ident_b[:], rhs=ident_b[:],
                                     start=True, stop=True)
                wa = const.tile([P, 16], fp32)
                nc.vector.tensor_copy(out=warm[:, 0:16], in_=pw[:, 0:16])
                nc.scalar.activation(out=wa[:], in_=pw[:, 0:16], func=AF.Relu)
                nc.scalar.activation(out=wa[:], in_=pw[:, 0:16], func=AF.Exp)
                nc.scalar.activation(out=wa[:], in_=pw[:, 0:16], func=AF.Ln)

            imgT8 = big.tile([P, 4, BATCH], f8)     # ALL image encodings (local MLP)
            M_s = big.tile([P, 2, BATCH], f8)       # M = W_g2^T-fused image side
            diag_s = stat.tile([P, RT], fp32)
            contrib = stat.tile([P, RT], fp32)

            # ===== image MLP, replicated over ALL 4096 images =================
            # n-tile 0 (this core's own images) runs first; tiles 1-7 are
            # emitted AFTER the h AllGather trigger so TensorE fills the
            # otherwise-idle mesh-wait window
            h1t = big.tile([P, 2, BATCH], f8)

            def mlp_tile(n, pool, tag1, tag2):
                sl = slice(n * 512, (n + 1) * 512)
                for m in range(2):
                    pm = pool.tile([P, 512], fp32, tag=tag1)
                    for k in range(4):
                        nc.tensor.matmul(
                            out=pm[:], lhsT=wi1_s[:, k, m, :], rhs=imt_s[:, k, sl],
                            start=(k == 0), stop=(k == 3),
                        )
                    nc.scalar.activation(
                        out=h1t[:, m, sl], in_=pm[:], func=AF.Relu,
                        bias=bi1_s[:, m:m + 1], scale=1.0,
                    )
                for m in range(4):
                    pm2 = pool.tile([P, 512], fp32, tag=tag2)
                    for k in range(2):
                        nc.tensor.matmul(
                            out=pm2[:], lhsT=wi2_s[:, k, m, :], rhs=h1t[:, k, sl],
                            start=(k == 0), stop=(k == 1),
                        )
                    nc.scalar.activation(
                        out=imgT8[:, m, sl], in_=pm2[:], func=AF.Relu,
                        bias=bi2_s[:, m:m + 1], scale=1.0,
                    )

            # ===== GCN layer 1: all slot gathers issued up front ==============
            # two pieces per slot (<=9 tiles each fits the SWDGE ring), pieces
            # round-robined over the 4 queues so all generators stay fed; the
            # image AllGather is slipped in after round 2 (the engine would be
            # blocked on queue backpressure then anyway)
            ghs = []          # per slot: two piece tiles (piece-granular deps)
            pieces = []
            for s in range(CPC):
                half = (T1[s] + 1) // 2
                gha = big.tile([P, half, D], f8, name=f"gh{s}a")
                ghb = big.tile([P, T1[s] - half, D], f8, name=f"gh{s}b")
                pieces.append((s, 0, half, gha))
                pieces.append((s, half, T1[s], ghb))
                ghs.append((gha, ghb, half))
            for qn, (s, a, b, gt) in enumerate(pieces):  # noqa: B007
                nc.gpsimd.dma_gather(
                    out_ap=gt[:, :, :], in_ap=t_xrow[:, :],
                    idxs_ap=gidx1_s[:, (o1[s] + a) * 8:(o1[s] + b) * 8],
                    num_idxs=(b - a) * P, num_idxs_reg=(b - a) * P,
                    elem_size=D, single_packet=False,
                    queue_num=qn % 4,
                )

            with tc.tile_pool(name="ps_l1", bufs=2, space="PSUM") as ps_l1:
                for n in range(NT // 2):
                    mlp_tile(n, ps_l1, "agg1", "agg1")

                def l1_slot(s):
                    pa = ps_l1.tile([P, D], fp32, tag="agg1", name="pa")
                    gha, ghb, half = ghs[s]
                    for (gt, a, b) in ((gha, 0, half), (ghb, half, T1[s])):
                        j = a
                        while j < b:
                            if j + 1 < b:
                                nc.tensor.matmul(
                                    out=pa[:], lhsT=cm1_s[:, o1[s] + j:o1[s] + j + 2, :],
                                    rhs=gt[:, j - a:j - a + 2, :],
                                    start=(j == 0), stop=(j + 2 == T1[s]), perf_mode=DR,
                                )
                                j += 2
                            else:
                                nc.tensor.matmul(
                                    out=pa[:], lhsT=cm1_s[:, o1[s] + j, :],
                                    rhs=gt[:, j - a, :],
                                    start=(j == 0), stop=(j + 1 == T1[s]),
                                )
                                j += 1
                    a1 = work.tile([P, D], bf16, tag="a1", name="a1")
                    nc.vector.tensor_copy(out=a1[:], in_=pa[:])
                    a1t = work.tile([P, 4, P], bf16, tag="a1t", name="a1t")
                    for k in range(4):
                        pt1 = ps_l1.tile([P, P], bf16, tag="tps1", name="pt1")
                        nc.tensor.transpose(
                            out=pt1[:], in_=a1[:, k * P:(k + 1) * P], identity=ident_b[:]
                        )
                        nc.vector.tensor_copy(out=a1t[:, k, :], in_=pt1[:])
                    ph = ps_l1.tile([P, Hdim], fp32, tag="hps", name="ph")
                    for k in range(4):
                        nc.tensor.matmul(
                            out=ph[:], lhsT=a1t[:, k, :], rhs=wg1_s[:, k, :],
                            start=(k == 0), stop=False,
                        )
                    nc.tensor.matmul(
                        out=ph[:], lhsT=ones_row[:], rhs=bg1_s[:],
                        start=False, stop=True, skip_group_check=True,
                    )
                    h_sb = work.tile([P, Hdim], f8, tag="h_sb", name="h_sb")
                    nc.scalar.activation(out=h_sb[:], in_=ph[:], func=AF.Relu)
                    nc.sync.dma_start(out=h_own[:, s, :], in_=h_sb[:])

                for s in range(CPC):
                    l1_slot(s)
                nc.gpsimd.collective_compute(
                    "AllGather", mybir.AluOpType.bypass, replica_groups=rg,
                    ins=[h_own[:, :, :]], outs=[h_t[:, :]],
                )
                for n in range(NT // 2, NT):
                    mlp_tile(n, ps_l1, "agg1", "agg1")
                # M[h, j] = sum_d W_g2[h? -- see layout] : fuse W_g2 into the
                # image side so logits contract over Hdim=256 instead of D=512
                if b2nz:
                    bterm_s = big.tile([1, BATCH], fp32)
                for n in range(NT):
                    sl = slice(n * 512, (n + 1) * 512)
                    for k in range(2):
                        pmm = ps_l1.tile([P, 512], fp32, tag="agg1")
                        for d in range(4):
                            nc.tensor.matmul(
                                out=pmm[:], lhsT=wg2_s[:, d, k, :],
                                rhs=imgT8[:, d, sl],
                                start=(d == 0), stop=(d == 3),
                            )
                        nc.vector.tensor_copy(out=M_s[:, k, sl], in_=pmm[:])
                    if b2nz:
                        pbt = ps_l1.tile([1, 512], fp32, tag="hps")
                        for d in range(4):
                            nc.tensor.matmul(
                                out=pbt[:], lhsT=b2c_s[:, d:d + 1],
                                rhs=imgT8[:, d, sl],
                                start=(d == 0), stop=(d == 3),
                            )
                        nc.vector.tensor_copy(out=bterm_s[:, sl], in_=pbt[:])

            # ===== GCN layer 2 + logits/LSE ===================================
            T2m = max(T2)
            with ExitStack() as c2:
                ps_l2 = c2.enter_context(
                    tc.tile_pool(name="ps_l2", bufs=1, space="PSUM"))
                ps_lg = c2.enter_context(
                    tc.tile_pool(name="ps_lg", bufs=4, space="PSUM"))

                # gathers (2 pieces per row tile, round-robin queues,
                # piece-granular tiles so r0's MMs start after round 1)
                ga_t = []
                qn = 0
                for r in range(RT):
                    half = (T2[r] + 1) // 2
                    gaa = big.tile([P, half, Hdim], f8, name=f"ga{r}a")
                    gab = big.tile([P, T2[r] - half, Hdim], f8, name=f"ga{r}b")
                    for (a, b, gt) in ((0, half, gaa), (half, T2[r], gab)):
                        nc.gpsimd.dma_gather(
                            out_ap=gt[:, :, :], in_ap=h_t[:, :],
                            idxs_ap=gidx2_s[:, (off2[r] + a) * 8:(off2[r] + b) * 8],
                            num_idxs=(b - a) * P, num_idxs_reg=(b - a) * P,
                            elem_size=Hdim, single_packet=False,
                            queue_num=qn % 4,
                        )
                        qn += 1
                    ga_t.append((gaa, gab, half))

                # per row tile: aggregation, txtT, diag, logits + LSE
                for r in range(RT):
                    pa2 = ps_l2.tile([P, Hdim], fp32, tag="agg2", name="pa2", bufs=2)
                    gaa, gab, half = ga_t[r]
                    for j in range(T2[r]):
                        gt, jj = (gaa, j) if j < half else (gab, j - half)
                        nc.tensor.matmul(
                            out=pa2[:], lhsT=cm2_s[:, off2[r] + j, :],
                            rhs=gt[:, jj, :],
                            start=(j == 0), stop=(j + 1 == T2[r]),
                        )
                    a2 = work.tile([P, Hdim], bf16, tag="a2")
                    nc.vector.tensor_copy(out=a2[:], in_=pa2[:])
                    a2t8 = work.tile([P, 2, P], f8, tag="a2t")
                    for k in range(2):
                        pt = ps_l2.tile([P, P], bf16, tag="tps")
                        nc.tensor.transpose(
                            out=pt[:], in_=a2[:, k * P:(k + 1) * P], identity=ident_b[:]
                        )
                        nc.vector.tensor_copy(out=a2t8[:, k, :], in_=pt[:])
                    # diag = diagonal of (a2t8^T @ M_own): identity-mask + row sum
                    px = ps_l2.tile([P, P], fp32, tag="ptx")
                    for k in range(2):
                        nc.tensor.matmul(
                            out=px[:], lhsT=a2t8[:, k, :],
                            rhs=M_s[:, k, r * P:(r + 1) * P],
                            start=(k == 0), stop=(k == 1),
                        )
                    xs = work.tile([P, P], bf16, tag="xs")
                    nc.vector.tensor_copy(out=xs[:], in_=px[:])
                    xm = work.tile([P, P], bf16, tag="xm")
                    nc.vector.tensor_tensor(
                        out=xm[:], in0=xs[:], in1=ident_b[:],
                        op=mybir.AluOpType.mult,
                    )
                    nc.vector.reduce_sum(out=diag_s[:, r:r + 1], in_=xm[:], axis=AX.X)

                    # logits row-tile x full imgT, fused exp+accumulate
                    sums = stat.tile([P, NT], fp32, tag="sums")
                    if stable:
                        banks = []
                    for n in range(NT):
                        pl = ps_lg.tile([P, 512], fp32, tag="lg")
                        nc.tensor.matmul(
                            out=pl[:], lhsT=a2t8[:, :, :],
                            rhs=M_s[:, :, n * 512:(n + 1) * 512],
                            start=True, stop=not b2nz, perf_mode=DR,
                        )
                        if b2nz:
                            nc.tensor.matmul(
                                out=pl[:], lhsT=ones_row[:],
                                rhs=bterm_s[:, n * 512:(n + 1) * 512],
                                start=False, stop=True, skip_group_check=True,
                            )
                        if stable:
                            banks.append(pl)
                        else:
                            esc = work.tile([P, 512], bf16, tag="esc", bufs=1)
                            nc.scalar.activation(
                                out=esc[:], in_=pl[:], func=AF.Exp,
                                accum_out=sums[:, n:n + 1],
                            )
                    if stable:
                        maxes = stat.tile([P, NT], fp32, tag="maxes")
                        for n in range(NT):
                            nc.vector.reduce_max(out=maxes[:, n:n + 1], in_=banks[n][:], axis=AX.X)
                        rmax = stat.tile([P, 1], fp32, tag="rmax")
                        nc.vector.reduce_max(out=rmax[:], in_=maxes[:], axis=AX.X)
                        nrmax = stat.tile([P, 1], fp32, tag="nrmax")
                        nc.scalar.mul(nrmax[:], rmax[:], -1.0)
                        for n in range(NT):
                            esc = work.tile([P, 512], bf16, tag="esc")
                            nc.scalar.activation(
                                out=esc[:], in_=banks[n][:], func=AF.Exp,
                                bias=nrmax[:], scale=1.0, accum_out=sums[:, n:n + 1],
                            )
                    ssum = stat.tile([P, 1], fp32, tag="ssum")
                    nc.vector.reduce_sum(out=ssum[:], in_=sums[:], axis=AX.X)
                    lns = stat.tile([P, 1], fp32, tag="lns")
                    nc.scalar.activation(out=lns[:], in_=ssum[:], func=AF.Ln)
                    t1 = stat.tile([P, 1], fp32, tag="t1")
                    if stable:
                        nc.vector.tensor_add(out=t1[:], in0=rmax[:], in1=lns[:])
                        nc.vector.tensor_sub(out=t1[:], in0=t1[:], in1=diag_s[:, r:r + 1])
                    else:
                        nc.vector.tensor_sub(out=t1[:], in0=lns[:], in1=diag_s[:, r:r + 1])
                    nc.vector.tensor_mul(
                        out=contrib[:, r:r + 1], in0=t1[:], in1=labf_s[:, r:r + 1]
                    )

            rsum = stat.tile([P, 1], fp32, tag="rsum")
            nc.vector.reduce_sum(out=rsum[:], in_=contrib[:], axis=AX.X)
            with tc.tile_pool(name="ps_fin", bufs=1, space="PSUM") as ps_fin:
                pf = ps_fin.tile([1, 1], fp32)
                nc.tensor.matmul(out=pf[:], lhsT=rsum[:], rhs=ones_col[:], start=True, stop=True)
                fin = stat.tile([1, 1], fp32, tag="fin")
                nc.vector.tensor_copy(out=fin[:], in_=pf[:])
            nc.sync.dma_start(out=t_out[:], in_=fin[:])

    nc.compile()
    return nc


_CACHE = {}


def kernel(**inputs) -> np.ndarray:
    from concourse.bass_utils import run_bass_kernel_spmd

    shared, percore, key = _prep(inputs)
    ckey = (key[0], key[1], key[2], key[3])
    if ckey not in _CACHE:
        _CACHE[ckey] = _build(ckey)
    nc = _CACHE[ckey]

    in_maps = []
    for c in range(NCORES):
        m = dict(shared)
        pc = percore[c]
        m.update({"cmat1": pc["cmat1"], "gidx1": pc["gidx1"],
                  "cmat2": pc["cmat2"], "gidx2": pc["gidx2"],
                  "imt": pc["imt"], "labf": pc["labf"]})
        in_maps.append(m)

    trace = bool(int(os.environ.get("KERNEL_TRACE", "0")))
    try:
        res = run_bass_kernel_spmd(nc, in_maps, core_ids=list(range(NCORES)), trace=trace)
    except Exception:
        # transient NRT/device hiccups have been observed to clear on retry
        res = run_bass_kernel_spmd(nc, in_maps, core_ids=list(range(NCORES)), trace=trace)
    kernel.last_results = res
    total = sum(float(r["partial"][0, 0]) for r in res.results)
    return np.float32(total / BATCH + 1.0)


# revision 37
# speedup vs baseline: 1.0901x; 1.0482x over previous
"""Trainium2 Bass kernel for nn_CLIP_GCN_Model (2-layer GCN + MLP + contrastive loss).

Reformulation (validated numerically):
  out = mean_i(label_i * (lse_i - logits_ii)) + 1.0
(the triplet term of the reference is identically 1.0).

GCN layer: out = S @ (x @ W) + b where S = D^-1/2 (A+I) D^-1/2.

Structure (single collective, eager gather streams):
  1. L1 runs over all 10240 (padded) nodes: 80 dst-chunks of 128 nodes,
     balanced to (core, slot); per chunk the distinct source x rows (512B fp8)
     are dma_gathered and aggregated with a fp8 coefficient matrix C
     (DoubleRow matmuls in PSUM), then transposed, x W_g1 + bias + relu -> h.
     All 20 gather pieces (2 per slot, <=9 tiles each to fit the SWDGE ring)
     are dispatched up front, round-robined over the 4 queues so all four
     descriptor generators run concurrently.
  2. h slots are written p-major ([128, 10, 256] per rank; node row =
     rank*1280 + p*10 + slot) and shared with ONE AllGather -> h_t.
  3. The image MLP is fully replicated: every core encodes ALL 4096 images
     (fp8, column-rolled so its own 512 images sit in block 0 -- the row-wise
     LSE is invariant to logits column order). It is emitted AFTER the h
     AllGather trigger so TensorE fills the otherwise-idle mesh-wait window,
     and it removes the image AllGather entirely. A small warm-up block ramps
     the PE p-state / scalar activation tables before the first L1 slot.
  4. W_g2 is fused into the image side: M = W_g2-contracted image encodings
     [256, 4096] is computed once per core in the AllGather window, so each
     128-label row tile only needs its h aggregation [128, 256], a transpose,
     and ONE fp8 DoubleRow matmul per 512-column tile (K=256 instead of 512).
     The diagonal is extracted as diag(a2t^T @ M_own) via identity mask +
     row reduce. LSE uses fused exp+accumulate (fast path: a host-side bound
     check shows exp cannot overflow in this data regime).
"""

import os
import numpy as np
import ml_dtypes

BF16 = ml_dtypes.bfloat16
F8 = ml_dtypes.float8_e4m3   # TRN fp8e4 (max 240)

N_NODES = 10000
NPAD = 10240
D = 512
Hdim = 256
BATCH = 4096
NCORES = 8
P = 128
NCHUNK = NPAD // P          # 80
CPC = NCHUNK // NCORES      # 10 slots per core
NPC = NPAD // NCORES        # 1280 nodes per core
MPC = NPC // P              # 10 m-tiles per core
RT = 4                      # row tiles per core (512 rows each core)
NT = BATCH // 512           # 8 column tiles of 512
H5 = CPC // 2               # 5 slots per h-half
HALF_N = NPAD // 2          # 5120


def _wrap16(idx, n):
    """Layout indices for dma_gather: element i -> [i%16, i//16], replicated to 128 partitions."""
    assert len(idx) == n and n % 16 == 0
    base = idx.astype(np.int16).reshape(n // 16, 16).T  # [16, n/16]
    return np.ascontiguousarray(np.tile(base, (8, 1)))  # [128, n/16]


def _prep(inputs):
    """Host-side layout/sharding prep."""
    x = np.asarray(inputs["x_nodes"], dtype=np.float32)
    image = np.asarray(inputs["image"], dtype=np.float32)
    ei = np.asarray(inputs["edge_index"]).astype(np.int64)
    label = np.asarray(inputs["label"]).astype(np.int64)
    src, dst = ei[0], ei[1]

    deg = np.ones(N_NODES, np.float32)
    np.add.at(deg, dst, 1.0)
    dinv = (1.0 / np.sqrt(deg)).astype(np.float32)

    # in-edges grouped by dst (sorted once)
    order = np.argsort(dst, kind="stable")
    src_s, dst_s = src[order], dst[order]
    bound = np.searchsorted(dst_s, np.arange(N_NODES + 1))

    nn = np.arange(NPAD)

    # ---------------- L1: per-chunk dedup + balanced (core,slot) assignment --
    chunk_src = []      # distinct sources per chunk
    chunk_C = []        # [n_distinct, 128] fp32 coef
    for c in range(NCHUNK):
        n0, n1 = c * P, min((c + 1) * P, N_NODES)
        if n0 >= N_NODES:
            chunk_src.append(np.zeros(1, np.int64))
            chunk_C.append(np.zeros((1, P), np.float32))
            continue
        e0, e1 = bound[n0], bound[n1]
        es, ed = src_s[e0:e1], dst_s[e0:e1]
        selfn = np.arange(n0, n1)
        all_s = np.concatenate([es, selfn])
        all_d = np.concatenate([ed, selfn]) - n0
        coef = np.concatenate([dinv[es] * dinv[ed], dinv[selfn] ** 2])
        uniq, inv = np.unique(all_s, return_inverse=True)
        C = np.zeros((len(uniq), P), np.float32)
        np.add.at(C, (inv, all_d), coef)
        chunk_src.append(uniq)
        chunk_C.append(C)

    counts = np.array([len(s) for s in chunk_src])
    rank = np.argsort(-counts, kind="stable")
    a_k = np.zeros(NCHUNK, np.int64)   # chunk -> core
    s_k = np.zeros(NCHUNK, np.int64)   # chunk -> slot
    T1 = []
    for s in range(CPC):
        grp = rank[s * NCORES:(s + 1) * NCORES]
        a_k[grp] = np.arange(NCORES)
        s_k[grp] = s
        T1.append(int(np.ceil(counts[grp].max() / P)))
    T1 = tuple(T1)
    ST1 = sum(T1)
    off1 = np.concatenate([[0], np.cumsum(T1)])

    # node -> h gather row: chunk k=(n//128) at (core a, slot s), partition
    # p=n%128; h tiles are [128, 10, 256] per rank -> row a*1280 + p*10 + s.
    kk = nn // P
    pp_ = nn % P
    hrow = a_k[kk] * NPC + pp_ * CPC + s_k[kk]

    gidx1 = np.zeros((NCORES, P, ST1 * 8), np.int16)
    cmat1 = np.zeros((NCORES, P, ST1, P), F8)
    for c in range(NCHUNK):
        cr, sl = a_k[c], s_k[c]
        E_s = T1[sl] * P
        idxs = np.zeros(E_s, np.int64)
        idxs[:counts[c]] = chunk_src[c]
        gidx1[cr, :, off1[sl] * 8:off1[sl + 1] * 8] = _wrap16(idxs, E_s)
        Cp = np.zeros((E_s, P), np.float32)
        Cp[:counts[c]] = chunk_C[c]
        # edge-slot e -> [partition e%128, tile e//128]
        cmat1[cr, :, off1[sl]:off1[sl + 1], :] = \
            Cp.reshape(T1[sl], P, P).transpose(1, 0, 2).astype(F8)

    # ---------------- L2: per-row-tile (labeled dst only), single phase ------
    bins = label.reshape(NCORES, RT, P)   # core c, tile r, row p -> label node
    t2 = np.zeros((NCORES, RT), np.int64)
    binsrc = {}
    for c in range(NCORES):
        for r in range(RT):
            labs = bins[c, r]
            segs, segd, segc = [], [], []
            for p in range(P):
                v = labs[p]
                e0, e1 = bound[v], bound[v + 1]
                es = src_s[e0:e1]
                segs.append(np.concatenate([es, [v]]))
                segd.append(np.full(len(es) + 1, p, np.int64))
                segc.append(np.concatenate([dinv[es] * dinv[v], [dinv[v] ** 2]]))
            all_s = np.concatenate(segs)
            all_d = np.concatenate(segd)
            coef = np.concatenate(segc)
            hr = hrow[all_s]
            uniq, inv = np.unique(hr, return_inverse=True)
            C = np.zeros((len(uniq), P), np.float32)
            np.add.at(C, (inv, all_d), coef)
            t2[c, r] = int(np.ceil(len(uniq) / P))
            binsrc[(c, r)] = (uniq, C)
    T2 = tuple(int(t2[:, r].max()) for r in range(RT))
    ST2 = sum(T2)
    off2 = np.concatenate([[0], np.cumsum(T2)])

    gidx2 = np.zeros((NCORES, P, ST2 * 8), np.int16)
    cmat2 = np.zeros((NCORES, P, ST2, P), F8)
    for c in range(NCORES):
        for r in range(RT):
            uniq, C = binsrc[(c, r)]
            Ea = T2[r] * P
            ia = np.zeros(Ea, np.int64)
            ia[:len(uniq)] = uniq
            gidx2[c, :, off2[r] * 8:off2[r + 1] * 8] = _wrap16(ia, Ea)
            Ca = np.zeros((Ea, P), np.float32)
            Ca[:len(uniq)] = C
            cmat2[c, :, off2[r]:off2[r + 1], :] = \
                Ca.reshape(T2[r], P, P).transpose(1, 0, 2).astype(F8)

    # ---------------- softmax-stability bound (cheap fp32 host forward) ------
    def _agg_all(xw):
        # fast segment sum via reduceat on the dst-sorted edges
        msg = (dinv[src_s] * dinv[dst_s])[:, None] * xw[src_s]
        agg = np.zeros_like(xw)
        nz = np.flatnonzero(np.diff(np.append(-1, dst_s)))
        agg[dst_s[nz]] = np.add.reduceat(msg, nz, axis=0)
        return agg + (dinv * dinv)[:, None] * xw

    h_np = np.maximum(_agg_all(x @ np.asarray(inputs["W_g1"], np.float32))
                      + np.asarray(inputs["b_g1"], np.float32), 0.0)
    g_np = _agg_all(h_np @ np.asarray(inputs["W_g2"], np.float32)) \
        + np.asarray(inputs["b_g2"], np.float32)
    img_np = np.maximum(image @ np.asarray(inputs["W_img1"], np.float32)
                        + np.asarray(inputs["b_img1"], np.float32), 0.0)
    img_np = np.maximum(img_np @ np.asarray(inputs["W_img2"], np.float32)
                        + np.asarray(inputs["b_img2"], np.float32), 0.0)
    bnd_logit = float(np.linalg.norm(g_np[label], axis=1).max()
                      * np.linalg.norm(img_np, axis=1).max())
    stable = bnd_logit > 60.0
    b2nz = bool(np.any(np.asarray(inputs["b_g2"], np.float32)))

    # ---------------- tensors ------------------------------------------------
    xpad = np.zeros((NPAD, D), np.float32)
    xpad[:N_NODES] = x
    xrow = np.ascontiguousarray(xpad).astype(F8)

    def km(w, kt):  # [K, M] -> [128p, kt, M]
        return np.ascontiguousarray(
            w.reshape(kt, P, w.shape[1]).transpose(1, 0, 2)
        ).astype(BF16)

    shared = {
        "xrow": xrow,
        "wg1": km(np.asarray(inputs["W_g1"], np.float32), 4),       # [128, 4, 256]
        "wg2k": np.ascontiguousarray(
            np.asarray(inputs["W_g2"], np.float32).reshape(2, P, 4, P).transpose(3, 2, 0, 1)
        ).astype(BF16),                                             # [128d, 4dblk, 2k, 128h]
        "wi1": np.ascontiguousarray(
            np.asarray(inputs["W_img1"], np.float32).reshape(4, P, 2, P).transpose(1, 0, 2, 3)
        ).astype(BF16),
        "wi2": np.ascontiguousarray(
            np.asarray(inputs["W_img2"], np.float32).reshape(2, P, 4, P).transpose(1, 0, 2, 3)
        ).astype(BF16),
        "bg1": np.asarray(inputs["b_g1"], np.float32).astype(BF16).reshape(1, Hdim),
        "bg2": np.asarray(inputs["b_g2"], np.float32).astype(BF16).reshape(1, D),
        "bi1": np.ascontiguousarray(np.asarray(inputs["b_img1"], np.float32).reshape(2, P).T),
        "bi2": np.ascontiguousarray(np.asarray(inputs["b_img2"], np.float32).reshape(4, P).T),
    }

    imageb = image.astype(np.float32)
    percore = []
    for c in range(NCORES):
        # all 4096 images, rolled so this core's own 512 come first (column
        # order of the logits is irrelevant to the row-wise LSE)
        rolled = np.concatenate([imageb[c * 512:], imageb[:c * 512]], axis=0)
        imt = np.ascontiguousarray(
            rolled.T.reshape(4, P, BATCH).transpose(1, 0, 2)
        ).astype(F8)  # [128 kpart, 4 kblk, 4096 imgs]
        labf = np.ascontiguousarray(
            label[c * 512:(c + 1) * 512].astype(np.float32).reshape(RT, P).T
        )  # [128, RT]
        percore.append({
            "cmat1": np.ascontiguousarray(cmat1[c]),
            "gidx1": np.ascontiguousarray(gidx1[c]),
            "cmat2": np.ascontiguousarray(cmat2[c]),
            "gidx2": np.ascontiguousarray(gidx2[c]),
            "imt": imt, "labf": labf,
        })
    shared["b2c"] = np.ascontiguousarray(
        np.asarray(inputs["b_g2"], np.float32).reshape(4, P).T)   # [128, 4]
    return shared, percore, (T1, T2, stable, b2nz)


def _build(key):
    """Build the SPMD Bass program."""
    T1, T2, stable, b2nz = key
    import concourse.bass as bass  # noqa: F401
    import concourse.tile as tile
    from concourse import bacc, mybir
    from concourse.masks import make_identity

    fp32 = mybir.dt.float32
    bf16 = mybir.dt.bfloat16
    f8 = mybir.dt.float8e4
    i16 = mybir.dt.int16
    AF = mybir.ActivationFunctionType
    AX = mybir.AxisListType
    DR = mybir.MatmulPerfMode.DoubleRow
    ST1 = sum(T1)
    ST2 = sum(T2)
    o1 = [0]
    for t in T1:
        o1.append(o1[-1] + t)
    off2 = [0]
    for t in T2:
        off2.append(off2[-1] + t)

    nc = bacc.Bacc("TRN2", target_bir_lowering=False, debug=False,
                   num_devices=NCORES, num_swdge_queues=4)

    t_xrow = nc.dram_tensor("xrow", [NPAD, D], f8, kind="ExternalInput").ap()
    t_wg1 = nc.dram_tensor("wg1", [P, 4, Hdim], bf16, kind="ExternalInput").ap()
    t_wg2k = nc.dram_tensor("wg2k", [P, 4, 2, P], bf16, kind="ExternalInput").ap()
    t_b2c = nc.dram_tensor("b2c", [P, 4], fp32, kind="ExternalInput").ap()
    t_wi1 = nc.dram_tensor("wi1", [P, 4, 2, P], bf16, kind="ExternalInput").ap()
    t_wi2 = nc.dram_tensor("wi2", [P, 2, 4, P], bf16, kind="ExternalInput").ap()
    t_bg1 = nc.dram_tensor("bg1", [1, Hdim], bf16, kind="ExternalInput").ap()
    t_bg2 = nc.dram_tensor("bg2", [1, D], bf16, kind="ExternalInput").ap()
    t_bi1 = nc.dram_tensor("bi1", [P, 2], fp32, kind="ExternalInput").ap()
    t_bi2 = nc.dram_tensor("bi2", [P, 4], fp32, kind="ExternalInput").ap()
    t_cmat1 = nc.dram_tensor("cmat1", [P, ST1, P], f8, kind="ExternalInput").ap()
    t_gidx1 = nc.dram_tensor("gidx1", [P, ST1 * 8], i16, kind="ExternalInput").ap()
    t_cmat2 = nc.dram_tensor("cmat2", [P, ST2, P], f8, kind="ExternalInput").ap()
    t_gidx2 = nc.dram_tensor("gidx2", [P, ST2 * 8], i16, kind="ExternalInput").ap()
    t_imt = nc.dram_tensor("imt", [P, 4, BATCH], f8, kind="ExternalInput").ap()
    t_labf = nc.dram_tensor("labf", [P, RT], fp32, kind="ExternalInput").ap()
    t_out = nc.dram_tensor("partial", [1, 1], fp32, kind="ExternalOutput").ap()

    rg = [list(range(NCORES))]

    with tile.TileContext(nc) as tc:
        from contextlib import ExitStack
        with ExitStack() as ctx:
            dram = ctx.enter_context(tc.tile_pool(name="dram", bufs=1, space="DRAM"))
            const = ctx.enter_context(tc.tile_pool(name="const", bufs=1))
            big = ctx.enter_context(tc.tile_pool(name="big", bufs=1))
            work = ctx.enter_context(tc.tile_pool(name="work", bufs=3))
            stat = ctx.enter_context(tc.tile_pool(name="stat", bufs=4))

            h_own = dram.tile([P, CPC, Hdim], f8)               # h all 10 slots
            h_t = dram.tile([NPAD, Hdim], f8, addr_space="Shared")

            # ---- constants in SBUF: gather idx first on the sync queue ----
            gidx1_s = const.tile([P, ST1 * 8], i16)
            nc.sync.dma_start(out=gidx1_s[:], in_=t_gidx1[:])
            cm1_s = const.tile([P, ST1, P], f8)
            nc.scalar.dma_start(out=cm1_s[:], in_=t_cmat1[:])
            wi1_s = const.tile([P, 4, 2, P], bf16)
            nc.sync.dma_start(out=wi1_s[:], in_=t_wi1[:])
            wi2_s = const.tile([P, 2, 4, P], bf16)
            nc.sync.dma_start(out=wi2_s[:], in_=t_wi2[:])
            bi1_s = const.tile([P, 2], fp32)
            nc.sync.dma_start(out=bi1_s[:], in_=t_bi1[:])
            bi2_s = const.tile([P, 4], fp32)
            nc.sync.dma_start(out=bi2_s[:], in_=t_bi2[:])
            wg1_s = const.tile([P, 4, Hdim], bf16)
            nc.scalar.dma_start(out=wg1_s[:], in_=t_wg1[:])
            gidx2_s = const.tile([P, ST2 * 8], i16)
            nc.scalar.dma_start(out=gidx2_s[:], in_=t_gidx2[:])
            cm2_s = const.tile([P, ST2, P], f8)
            nc.scalar.dma_start(out=cm2_s[:], in_=t_cmat2[:])
            bg1_s = const.tile([1, Hdim], bf16)
            nc.scalar.dma_start(out=bg1_s[:], in_=t_bg1[:])
            bg2_s = const.tile([1, D], bf16)
            nc.scalar.dma_start(out=bg2_s[:], in_=t_bg2[:])
            wg2_s = const.tile([P, 4, 2, P], bf16)
            nc.scalar.dma_start(out=wg2_s[:], in_=t_wg2k[:])
            b2c_s = const.tile([P, 4], fp32)
            nc.scalar.dma_start(out=b2c_s[:], in_=t_b2c[:])
            labf_s = const.tile([P, RT], fp32)
            nc.scalar.dma_start(out=labf_s[:], in_=t_labf[:])
            imt_s = const.tile([P, 4, BATCH], f8)
            nc.scalar.dma_start(out=imt_s[:], in_=t_imt[:])
            ones_row = const.tile([1, P], bf16)
            nc.vector.memset(ones_row[:], 1.0)
            ones_cb = const.tile([P, 1], bf16)
            nc.vector.memset(ones_cb[:], 1.0)
            ones_col = const.tile([P, 1], fp32)
            nc.vector.memset(ones_col[:], 1.0)
            ident_b = const.tile([P, P], bf16)
            make_identity(nc, ident_b[:])

            # warm-up: ramp the PE p-state and preload scalar activation
            # tables while the input DMAs stream (otherwise the first L1 slot
            # chain pays the cold-clock + table-load penalty)
            warm = const.tile([P, P], bf16)
            with tc.tile_pool(name="ps_warm", bufs=1, space="PSUM") as ps_warm:
                pw = ps_warm.tile([P, P], fp32)
                for _ in range(12):
                    nc.tensor.matmul(out=pw[:], lhsT=ident_b[:], rhs=ident_b[:],
                                     start=True, stop=True)
                wa = const.tile([P, 16], fp32)
                nc.vector.tensor_copy(out=warm[:, 0:16], in_=pw[:, 0:16])
                nc.scalar.activation(out=wa[:], in_=pw[:, 0:16], func=AF.Relu)
                nc.scalar.activation(out=wa[:], in_=pw[:, 0:16], func=AF.Exp)
                nc.scalar.activation(out=wa[:], in_=pw[:, 0:16], func=AF.Ln)

            imgT8 = big.tile([P, 4, BATCH], f8)     # ALL image encodings (local MLP)
            M_s = big.tile([P, 2, BATCH], f8)       # M = W_g2^T-fused image side
            diag_s = stat.tile([P, RT], fp32)
            contrib = stat.tile([P, RT], fp32)

            # ===== image MLP, replicated over ALL 4096 images =================
            # n-tile 0 (this core's own images) runs first; tiles 1-7 are
            # emitted AFTER the h AllGather trigger so TensorE fills the
            # otherwise-idle mesh-wait window
            h1t = big.tile([P, 2, BATCH], f8)

            def mlp_tile(n, pool, tag1, tag2):
                sl = slice(n * 512, (n + 1) * 512)
                for m in range(2):
                    pm = pool.tile([P, 512], fp32, tag=tag1)
                    for k in range(4):
                        nc.tensor.matmul(
                            out=pm[:], lhsT=wi1_s[:, k, m, :], rhs=imt_s[:, k, sl],
                            start=(k == 0), stop=(k == 3),
                        )
                    nc.scalar.activation(
                        out=h1t[:, m, sl], in_=pm[:], func=AF.Relu,
                        bias=bi1_s[:, m:m + 1], scale=1.0,
                    )
                for m in range(4):
                    pm2 = pool.tile([P, 512], fp32, tag=tag2)
                    for k in range(2):
                        nc.tensor.matmul(
                            out=pm2[:], lhsT=wi2_s[:, k, m, :], rhs=h1t[:, k, sl],
                            start=(k == 0), stop=(k == 1),
                        )
                    nc.scalar.activation(
                        out=imgT8[:, m, sl], in_=pm2[:], func=AF.Relu,
                        bias=bi2_s[:, m:m + 1], scale=1.0,
                    )

            # ===== GCN layer 1: all slot gathers issued up front ==============
            # two pieces per slot (<=9 tiles each fits the SWDGE ring), pieces
            # round-robined over the 4 queues so all generators stay fed; the
            # image AllGather is slipped in after round 2 (the engine would be
            # blocked on queue backpressure then anyway)
            ghs = []          # per slot: two piece tiles (piece-granular deps)
            pieces = []
            for s in range(CPC):
                half = (T1[s] + 1) // 2
                gha = big.tile([P, half, D], f8, name=f"gh{s}a")
                ghb = big.tile([P, T1[s] - half, D], f8, name=f"gh{s}b")
                pieces.append((s, 0, half, gha))
                pieces.append((s, half, T1[s], ghb))
                ghs.append((gha, ghb, half))
            for qn, (s, a, b, gt) in enumerate(pieces):  # noqa: B007
                nc.gpsimd.dma_gather(
                    out_ap=gt[:, :, :], in_ap=t_xrow[:, :],
                    idxs_ap=gidx1_s[:, (o1[s] + a) * 8:(o1[s] + b) * 8],
                    num_idxs=(b - a) * P, num_idxs_reg=(b - a) * P,
                    elem_size=D, single_packet=False,
                    queue_num=qn % 4,
                )

            with tc.tile_pool(name="ps_l1", bufs=2, space="PSUM") as ps_l1:
                def l1_slot(s):
                    pa = ps_l1.tile([P, D], fp32, tag="agg1", name="pa")
                    gha, ghb, half = ghs[s]
                    for (gt, a, b) in ((gha, 0, half), (ghb, half, T1[s])):
                        j = a
                        while j < b:
                            if j + 1 < b:
                                nc.tensor.matmul(
                                    out=pa[:], lhsT=cm1_s[:, o1[s] + j:o1[s] + j + 2, :],
                                    rhs=gt[:, j - a:j - a + 2, :],
                                    start=(j == 0), stop=(j + 2 == T1[s]), perf_mode=DR,
                                )
                                j += 2
                            else:
                                nc.tensor.matmul(
                                    out=pa[:], lhsT=cm1_s[:, o1[s] + j, :],
                                    rhs=gt[:, j - a, :],
                                    start=(j == 0), stop=(j + 1 == T1[s]),
                                )
                                j += 1
                    a1 = work.tile([P, D], bf16, tag="a1", name="a1")
                    nc.vector.tensor_copy(out=a1[:], in_=pa[:])
                    a1t = work.tile([P, 4, P], bf16, tag="a1t", name="a1t")
                    for k in range(4):
                        pt1 = ps_l1.tile([P, P], bf16, tag="tps1", name="pt1")
                        nc.tensor.transpose(
                            out=pt1[:], in_=a1[:, k * P:(k + 1) * P], identity=ident_b[:]
                        )
                        nc.vector.tensor_copy(out=a1t[:, k, :], in_=pt1[:])
                    ph = ps_l1.tile([P, Hdim], fp32, tag="hps", name="ph")
                    for k in range(4):
                        nc.tensor.matmul(
                            out=ph[:], lhsT=a1t[:, k, :], rhs=wg1_s[:, k, :],
                            start=(k == 0), stop=False,
                        )
                    nc.tensor.matmul(
                        out=ph[:], lhsT=ones_row[:], rhs=bg1_s[:],
                        start=False, stop=True, skip_group_check=True,
                    )
                    h_sb = work.tile([P, Hdim], f8, tag="h_sb", name="h_sb")
                    nc.scalar.activation(out=h_sb[:], in_=ph[:], func=AF.Relu)
                    nc.sync.dma_start(out=h_own[:, s, :], in_=h_sb[:])

                for s in range(CPC):
                    l1_slot(s)
                nc.gpsimd.collective_compute(
                    "AllGather", mybir.AluOpType.bypass, replica_groups=rg,
                    ins=[h_own[:, :, :]], outs=[h_t[:, :]],
                )
                for n in range(NT):
                    mlp_tile(n, ps_l1, "agg1", "agg1")
                # M[h, j] = sum_d W_g2[h? -- see layout] : fuse W_g2 into the
                # image side so logits contract over Hdim=256 instead of D=512
                if b2nz:
                    bterm_s = big.tile([1, BATCH], fp32)
                for n in range(NT):
                    sl = slice(n * 512, (n + 1) * 512)
                    for k in range(2):
                        pmm = ps_l1.tile([P, 512], fp32, tag="agg1")
                        for d in range(4):
                            nc.tensor.matmul(
                                out=pmm[:], lhsT=wg2_s[:, d, k, :],
                                rhs=imgT8[:, d, sl],
                                start=(d == 0), stop=(d == 3),
                            )
                        nc.vector.tensor_copy(out=M_s[:, k, sl], in_=pmm[:])
                    if b2nz:
                        pbt = ps_l1.tile([1, 512], fp32, tag="hps")
                        for d in range(4):
                            nc.tensor.matmul(
                                out=pbt[:], lhsT=b2c_s[:, d:d + 1],
                                rhs=imgT8[:, d, sl],
                                start=(d == 0), stop=(d == 3),
                            )
                        nc.vector.tensor_copy(out=bterm_s[:, sl], in_=pbt[:])

            # ===== GCN layer 2 + logits/LSE ===================================
            T2m = max(T2)
            with ExitStack() as c2:
                ps_l2 = c2.enter_context(
                    tc.tile_pool(name="ps_l2", bufs=1, space="PSUM"))
                ps_lg = c2.enter_context(
                    tc.tile_pool(name="ps_lg", bufs=3, space="PSUM"))

                # gathers (2 pieces per row tile, round-robin queues,
                # piece-granular tiles so r0's MMs start after round 1)
                ga_t = []
                qn = 0
                for r in range(RT):
                    half = (T2[r] + 1) // 2
                    gaa = big.tile([P, half, Hdim], f8, name=f"ga{r}a")
                    gab = big.tile([P, T2[r] - half, Hdim], f8, name=f"ga{r}b")
                    for (a, b, gt) in ((0, half, gaa), (half, T2[r], gab)):
                        nc.gpsimd.dma_gather(
                            out_ap=gt[:, :, :], in_ap=h_t[:, :],
                            idxs_ap=gidx2_s[:, (off2[r] + a) * 8:(off2[r] + b) * 8],
                            num_idxs=(b - a) * P, num_idxs_reg=(b - a) * P,
                            elem_size=Hdim, single_packet=False,
                            queue_num=qn % 4,
                        )
                        qn += 1
                    ga_t.append((gaa, gab, half))

                # per row tile: aggregation, txtT, diag, logits + LSE
                for r in range(RT):
                    pa2 = ps_l2.tile([P, Hdim], fp32, tag="agg2", name="pa2", bufs=2)
                    gaa, gab, half = ga_t[r]
                    for j in range(T2[r]):
                        gt, jj = (gaa, j) if j < half else (gab, j - half)
                        nc.tensor.matmul(
                            out=pa2[:], lhsT=cm2_s[:, off2[r] + j, :],
                            rhs=gt[:, jj, :],
                            start=(j == 0), stop=(j + 1 == T2[r]),
                        )
                    a2 = work.tile([P, Hdim], bf16, tag="a2")
                    nc.vector.tensor_copy(out=a2[:], in_=pa2[:])
                    a2t8 = work.tile([P, 2, P], f8, tag="a2t")
                    for k in range(2):
                        pt = ps_l2.tile([P, P], bf16, tag="tps")
                        nc.tensor.transpose(
                            out=pt[:], in_=a2[:, k * P:(k + 1) * P], identity=ident_b[:]
                        )
                        nc.vector.tensor_copy(out=a2t8[:, k, :], in_=pt[:])
                    # diag = diagonal of (a2t8^T @ M_own): identity-mask + row sum
                    px = ps_l2.tile([P, P], fp32, tag="ptx")
                    for k in range(2):
                        nc.tensor.matmul(
                            out=px[:], lhsT=a2t8[:, k, :],
                            rhs=M_s[:, k, r * P:(r + 1) * P],
                            start=(k == 0), stop=(k == 1),
                        )
                    xs = work.tile([P, P], bf16, tag="xs")
                    nc.vector.tensor_copy(out=xs[:], in_=px[:])
                    xm = work.tile([P, P], bf16, tag="xm")
                    nc.vector.tensor_tensor(
                        out=xm[:], in0=xs[:], in1=ident_b[:],
                        op=mybir.AluOpType.mult,
                    )
                    nc.vector.reduce_sum(out=diag_s[:, r:r + 1], in_=xm[:], axis=AX.X)

                    # logits row-tile x full imgT, fused exp+accumulate
                    sums = stat.tile([P, NT], fp32, tag="sums")
                    if stable:
                        banks = []
                    for n in range(NT):
                        pl = ps_lg.tile([P, 512], fp32, tag="lg")
                        nc.tensor.matmul(
                            out=pl[:], lhsT=a2t8[:, :, :],
                            rhs=M_s[:, :, n * 512:(n + 1) * 512],
                            start=True, stop=not b2nz, perf_mode=DR,
                        )
                        if b2nz:
                            nc.tensor.matmul(
                                out=pl[:], lhsT=ones_row[:],
                                rhs=bterm_s[:, n * 512:(n + 1) * 512],
                                start=False, stop=True, skip_group_check=True,
                            )
                        if stable:
                            banks.append(pl)
                        else:
                            esc = work.tile([P, 512], bf16, tag="esc", bufs=1)
                            nc.scalar.activation(
                                out=esc[:], in_=pl[:], func=AF.Exp,
                                accum_out=sums[:, n:n + 1],
                            )
                    if stable:
                        maxes = stat.tile([P, NT], fp32, tag="maxes")
                        for n in range(NT):
                            nc.vector.reduce_max(out=maxes[:, n:n + 1], in_=banks[n][:], axis=AX.X)
                        rmax = stat.tile([P, 1], fp32, tag="rmax")
                        nc.vector.reduce_max(out=rmax[:], in_=maxes[:], axis=AX.X)
                        nrmax = stat.tile([P, 1], fp32, tag="nrmax")
                        nc.scalar.mul(nrmax[:], rmax[:], -1.0)
                        for n in range(NT):
                            esc = work.tile([P, 512], bf16, tag="esc")
                            nc.scalar.activation(
                                out=esc[:], in_=banks[n][:], func=AF.Exp,
                                bias=nrmax[:], scale=1.0, accum_out=sums[:, n:n + 1],
                            )
                    ssum = stat.tile([P, 1], fp32, tag="ssum")
                    nc.vector.reduce_sum(out=ssum[:], in_=sums[:], axis=AX.X)
                    lns = stat.tile([P, 1], fp32, tag="lns")
                    nc.scalar.activation(out=lns[:], in_=ssum[:], func=AF.Ln)
                    t1 = stat.tile([P, 1], fp32, tag="t1")
                    if stable:
                        nc.vector.tensor_add(out=t1[:], in0=rmax[:], in1=lns[:])
                        nc.vector.tensor_sub(out=t1[:], in0=t1[:], in1=diag_s[:, r:r + 1])
                    else:
                        nc.vector.tensor_sub(out=t1[:], in0=lns[:], in1=diag_s[:, r:r + 1])
                    nc.vector.tensor_mul(
                        out=contrib[:, r:r + 1], in0=t1[:], in1=labf_s[:, r:r + 1]
                    )

            rsum = stat.tile([P, 1], fp32, tag="rsum")
            nc.vector.reduce_sum(out=rsum[:], in_=contrib[:], axis=AX.X)
            with tc.tile_pool(name="ps_fin", bufs=1, space="PSUM") as ps_fin:
                pf = ps_fin.tile([1, 1], fp32)
                nc.tensor.matmul(out=pf[:], lhsT=rsum[:], rhs=ones_col[:], start=True, stop=True)
                fin = stat.tile([1, 1], fp32, tag="fin")
                nc.vector.tensor_copy(out=fin[:], in_=pf[:])
            nc.sync.dma_start(out=t_out[:], in_=fin[:])

    nc.compile()
    return nc


_CACHE = {}


def kernel(**inputs) -> np.ndarray:
    from concourse.bass_utils import run_bass_kernel_spmd

    shared, percore, key = _prep(inputs)
    ckey = (key[0], key[1], key[2], key[3])
    if ckey not in _CACHE:
        _CACHE[ckey] = _build(ckey)
    nc = _CACHE[ckey]

    in_maps = []
    for c in range(NCORES):
        m = dict(shared)
        pc = percore[c]
        m.update({"cmat1": pc["cmat1"], "gidx1": pc["gidx1"],
                  "cmat2": pc["cmat2"], "gidx2": pc["gidx2"],
                  "imt": pc["imt"], "labf": pc["labf"]})
        in_maps.append(m)

    trace = bool(int(os.environ.get("KERNEL_TRACE", "0")))
    try:
        res = run_bass_kernel_spmd(nc, in_maps, core_ids=list(range(NCORES)), trace=trace)
    except Exception:
        # transient NRT/device hiccups have been observed to clear on retry
        res = run_bass_kernel_spmd(nc, in_maps, core_ids=list(range(NCORES)), trace=trace)
    kernel.last_results = res
    total = sum(float(r["partial"][0, 0]) for r in res.results)
    return np.float32(total / BATCH + 1.0)
